# revision 1
# baseline (speedup 1.0000x reference)
"""Trainium2 Bass kernel: Based linear attention (poly feature map, causal, normalized).

Full inputs q,k,v: [1, 16, 4096, 16] fp32. Output: [1, 16, 4096, 16] fp32.
Sharding: 16 heads over 8 cores (2 heads/core); each head is independent.

Algorithm (per head): chunked quadratic-state linear attention.
  scores s = scale * q.k ; poly P = 1 + s + 0.5 s^2 = (chi_a . psi_k)^2 + 0.5
  with psi_k = [1, k] (17-dim), chi_a = [1, scale*q]/sqrt(2).
  Quadratic state T[(r,p), d'] = sum_j psi_k_r psi_k_p v'_d with v' = [v, 1]
  (channel 16 carries the normalizer z). Per 128-position chunk: the diagonal
  block is computed directly (matmul + ACT Square + fused (sq+0.5)*mask);
  the cross-chunk part contracts query features against the state
  (G = chi_a^T T, H = G * bcast(chi_a), num2 = 17-group reduce of H).
  The +0.5*prefix(v') term is folded in by doubling the state's (0,0) feature
  row (4 tiny accumulating matmuls). The state is replicated at partition
  bases {0,32,64,96} via a materialized 4x-replicated k-feature tensor so the
  per-chunk transposed feature tiles (4 chunks packed per PE transpose) can
  matmul it directly. The two heads of a core are processed interleaved with
  their elementwise work fused into paired wide ops (halves fixed overheads).
"""
import numpy as np
from contextlib import ExitStack

import concourse.bass as bass
import concourse.bacc as bacc
import concourse.tile as tile
import concourse.mybir as mybir
from bass_rust import add_dep_helper
from concourse.masks import make_identity, make_upper_triangular
from concourse.bass_utils import run_bass_kernel_spmd

B, H, S, D = 1, 16, 4096, 16
NCORES = 8
HPC = H // NCORES  # heads per core (2)
C = 128            # chunk (positions)
NCH = S // C       # 32 chunks
D1 = D + 1         # 17
F = D1 * D1        # 289
dt = mybir.dt.float32
bt = mybir.dt.bfloat16
SCALE = 1.0 / np.sqrt(D)
SC = SCALE / np.sqrt(2.0)
RT2I = 1.0 / np.sqrt(2.0)


def _fd(ap, offset_ap, dims):
    """AP on the same tensor as `ap`, partition dim kept, free dims replaced."""
    return bass.AP(tensor=ap.tensor, offset=offset_ap.offset, ap=[ap.ap[0]] + dims)


def _build_core(nc, pools, q_d, k_d, v_d, o_d):
    (ident, mask, trih), bulk, sb, st, ps128, psG, psN, psT = pools

    # ---- raw loads (both heads): [h, S, D] -> [128, h, NCH, D] ----
    qraw = bulk.tile([128, HPC, NCH, D], dt, tag="qraw")
    kraw = bulk.tile([128, HPC, NCH, D], dt, tag="kraw")
    vraw = bulk.tile([128, HPC, NCH, D], dt, tag="vraw")
    for h in range(HPC):
        nc.sync.dma_start(qraw[:, h], q_d[h].rearrange("(c p) d -> p c d", p=128))
        nc.sync.dma_start(kraw[:, h], k_d[h].rearrange("(c p) d -> p c d", p=128))
        nc.sync.dma_start(vraw[:, h], v_d[h].rearrange("(c p) d -> p c d", p=128))

    # ---- paired bulk feature tensors ----
    # kb: [1|k|0*15] per chunk; ab: [1/sqrt2|sc*q|0*15]; vb: [v|1]
    # kb4: [1|k|0*15] x4 replicated (state-update lhsT)
    kb = bulk.tile([128, HPC, NCH, 32], bt, tag="kb")
    ab = bulk.tile([128, HPC, NCH, 32], bt, tag="ab")
    vb = bulk.tile([128, HPC, NCH, D1], bt, tag="vb")
    kb4 = bulk.tile([128, HPC, NCH, 128], bt, tag="kb4")
    nc.gpsimd.memset(kb[:], 0.0)
    nc.gpsimd.memset(ab[:], 0.0)
    nc.gpsimd.memset(kb4[:], 0.0)
    nc.vector.memset(kb[:, :, :, 0:1], 1.0)
    nc.vector.memset(ab[:, :, :, 0:1], RT2I)
    nc.vector.memset(vb[:, :, :, D : D + 1], 1.0)
    nc.scalar.copy(kb[:, :, :, 1 : D + 1], kraw[:])
    nc.scalar.mul(ab[:, :, :, 1 : D + 1], qraw[:], SC)
    nc.gpsimd.tensor_copy(vb[:, :, :, 0:D], vraw[:])
    kb4_r = kb4[:].rearrange("p h c (r e) -> p h c r e", r=4)
    nc.vector.memset(kb4_r[:, :, :, :, 0:1], 1.0)
    for h in range(HPC):
        kraw_bc = bass.AP(tensor=kraw[:].tensor, offset=kraw[:, h].offset,
                          ap=[kraw[:].ap[0], [D, NCH], [0, 4], [1, D]])
        nc.vector.tensor_copy(kb4_r[:, h, :, :, 1 : D + 1], kraw_bc)

    # ---- transposed feature tiles: 4 chunks per [128,128] at bases {0,32,64,96} ----
    ktp = bulk.tile([128, HPC, NCH // 4, 128], bt, tag="ktp")
    atp = bulk.tile([128, HPC, NCH // 4, 128], bt, tag="atp")
    for h in range(HPC):
        for g in range(NCH // 4):
            kt_ps = ps128.tile([128, 2, 128], dt, tag="st")
            kb_slab = _fd(kb[:], kb[:, h, 4 * g, 0:1], [[1, 128]])
            ab_slab = _fd(ab[:], ab[:, h, 4 * g, 0:1], [[1, 128]])
            nc.tensor.matmul(kt_ps[:, 0, :], kb_slab, ident[:], start=True, stop=True)
            nc.tensor.matmul(kt_ps[:, 1, :], ab_slab, ident[:], start=True, stop=True)
            nc.scalar.copy(ktp[:, h, g, :], kt_ps[:, 0, :])
            nc.scalar.copy(atp[:, h, g, :], kt_ps[:, 1, :])

    o_sb = bulk.tile([128, HPC, NCH, D], dt, tag="osb")
    # paired state PSUM: head h at column offset 512*h (bank-aligned)
    t4p = psT.tile([128, HPC, 512], dt, tag="t4")
    nc.vector.memset(t4p[:], 0.0)
    t4sb_prev = None
    prev_copy = None

    for c in range(NCH):
        g, b = divmod(c, 4)
        p0 = 32 * b

        # paired intra scores: S_T[j, (h,q)]
        stp = ps128.tile([128, HPC, 128], dt, tag="st")
        for h in range(HPC):
            nc.tensor.matmul(stp[:, h, :], ktp[p0 : p0 + D1, h, g, :],
                             atp[p0 : p0 + D1, h, g, :],
                             start=True, stop=True, tile_position=(p0, 0))
        sq = sb.tile([128, HPC, 128], bt, tag="sq")
        nc.scalar.activation(sq[:], stp[:], mybir.ActivationFunctionType.Square)
        pt = sb.tile([128, HPC, 128], bt, tag="pt")
        mask_bc = _fd(mask[:], mask[:], [[0, HPC], [1, 128]])
        nc.gpsimd.tensor_mul(pt[:], sq[:], mask_bc)

        num_ps = psN.tile([128, HPC, D1], dt, tag="num")
        for h in range(HPC):
            m_pv = nc.tensor.matmul(num_ps[:, h, :], pt[:, h, :], vb[:, h, c, :],
                                    start=True, stop=False)
            m_tri = nc.tensor.matmul(num_ps[:, h, :], trih[:], vb[:, h, c, :],
                                     start=False, stop=True)
            add_dep_helper(m_tri.ins, m_pv.ins, reason="trih after pv start")

        tot = sb.tile([128, HPC, D1], dt, tag="tot")
        if c > 0:
            gp = psG.tile([128, HPC, 512], dt, tag="g")
            for h in range(HPC):
                nc.tensor.matmul(gp[:, h, 0:F], atp[p0 : p0 + D1, h, g, :],
                                 t4sb_prev[p0 : p0 + D1, h, :],
                                 start=True, stop=True, tile_position=(p0, 0))
            h_t = sb.tile([128, HPC, F], bt, tag="h")
            num2 = sb.tile([128, HPC, D1], dt, tag="num2")
            for h in range(HPC):
                ab_bc = _fd(ab[:], ab[:, h, c, 0:1], [[1, D1], [0, D1]])
                nc.vector.scalar_tensor_tensor(
                    h_t[:, h, :], gp[:, h, 0:F], 1.0, ab_bc,
                    mybir.AluOpType.mult, mybir.AluOpType.mult
                )
                h_r = _fd(h_t[:], h_t[:, h, 0:1], [[1, D1], [D1, D1]])
                nc.vector.tensor_reduce(num2[:, h, :], h_r,
                                        axis=mybir.AxisListType.X,
                                        op=mybir.AluOpType.add)
            nc.vector.scalar_tensor_tensor(
                tot[:], num_ps[:], 1.0, num2[:], mybir.AluOpType.mult,
                mybir.AluOpType.add
            )
        else:
            nc.scalar.copy(tot[:], num_ps[:])

        # normalize: out = num / z (z = channel 16)
        rec = sb.tile([128, HPC, 1], dt, tag="rec")
        nc.vector.reciprocal(rec[:], tot[:, :, D : D + 1])
        for h in range(HPC):
            nc.scalar.activation(o_sb[:, h, c, :], tot[:, h, 0:D],
                                 mybir.ActivationFunctionType.Copy,
                                 scale=rec[:, h, :])

        # state update: T += [psi_k | psi_k (x) W] per position, 4-base replicated
        w = sb.tile([128, HPC, D * D1], bt, tag="w")
        for h in range(HPC):
            kb_bc = _fd(kb[:], kb[:, h, c, 1 : 1 + D], [[1, D], [0, D1]])
            vb_bc = _fd(vb[:], vb[:, h, c, 0:1], [[0, D], [1, D1]])
            nc.gpsimd.tensor_mul(w[:, h, :], kb_bc, vb_bc)
        mms = []
        for h in range(HPC):
            mms.append(nc.tensor.matmul(t4p[:, h, 0:D1], kb4[:, h, c, :],
                                        vb[:, h, c, :], start=False, stop=False,
                                        skip_group_check=True))
            mms.append(nc.tensor.matmul(t4p[:, h, D1:F], kb4[:, h, c, :],
                                        w[:, h, :], start=False, stop=False,
                                        skip_group_check=True))
            for bb in range(4):
                mms.append(nc.tensor.matmul(
                    t4p[32 * bb : 32 * bb + 1, h, 0:D1],
                    kb4[:, h, c, 0:1], vb[:, h, c, :],
                    start=False, stop=False, tile_position=(0, 32 * bb),
                    skip_group_check=True))
        if prev_copy is not None:
            for m in mms:
                add_dep_helper(m.ins, prev_copy.ins, reason="t4 update after snapshot")

        if c < NCH - 1:
            t4sb = st.tile([128, HPC, F], bt, tag="t4sb")
            cp = nc.scalar.copy(t4sb[:], t4p[:, :, 0:F])
            for m in mms:
                add_dep_helper(cp.ins, m.ins, reason="snapshot after t4 update")
            t4sb_prev = t4sb
            prev_copy = cp

    for h in range(HPC):
        nc.sync.dma_start(o_d[h].rearrange("(c p) d -> p c d", p=128), o_sb[:, h])


def build_program():
    nc = bacc.Bacc("TRN2", target_bir_lowering=False, debug=False)
    q_d = nc.dram_tensor("q", [HPC, S, D], dt, kind="ExternalInput")
    k_d = nc.dram_tensor("k", [HPC, S, D], dt, kind="ExternalInput")
    v_d = nc.dram_tensor("v", [HPC, S, D], dt, kind="ExternalInput")
    o_d = nc.dram_tensor("out", [HPC, S, D], dt, kind="ExternalOutput")

    with tile.TileContext(nc) as tc, ExitStack() as ctx:
        constp = ctx.enter_context(tc.tile_pool(name="const", bufs=1))
        bulk = ctx.enter_context(tc.tile_pool(name="bulk", bufs=1))
        sb = ctx.enter_context(tc.tile_pool(name="sb", bufs=3))
        st = ctx.enter_context(tc.tile_pool(name="st", bufs=2))
        ps128 = ctx.enter_context(tc.tile_pool(name="ps128", bufs=2, space="PSUM"))
        psG = ctx.enter_context(tc.tile_pool(name="psG", bufs=1, space="PSUM"))
        psN = ctx.enter_context(tc.tile_pool(name="psN", bufs=2, space="PSUM"))
        psT = ctx.enter_context(tc.tile_pool(name="psT", bufs=1, space="PSUM"))

        ident = constp.tile([128, 128], bt)
        make_identity(nc, ident)
        mask = constp.tile([128, 128], bt)
        make_upper_triangular(nc, mask, val=1.0, diag=True)
        trih = constp.tile([128, 128], bt)
        make_upper_triangular(nc, trih, val=0.5, diag=True)

        pools = ((ident, mask, trih), bulk, sb, st, ps128, psG, psN, psT)
        _build_core(nc, pools, q_d, k_d, v_d, o_d)

    nc.compile()
    return nc


_NC = None


def kernel(q: np.ndarray, k: np.ndarray, v: np.ndarray) -> np.ndarray:
    global _NC
    if _NC is None:
        _NC = build_program()
    q = np.ascontiguousarray(np.asarray(q, dtype=np.float32).reshape(H, S, D))
    k = np.ascontiguousarray(np.asarray(k, dtype=np.float32).reshape(H, S, D))
    v = np.ascontiguousarray(np.asarray(v, dtype=np.float32).reshape(H, S, D))
    in_maps = []
    for i in range(NCORES):
        sl = slice(i * HPC, (i + 1) * HPC)
        in_maps.append({
            "q": np.ascontiguousarray(q[sl]),
            "k": np.ascontiguousarray(k[sl]),
            "v": np.ascontiguousarray(v[sl]),
        })
    res = run_bass_kernel_spmd(_NC, in_maps, core_ids=list(range(NCORES)))
    outs = [res.results[i]["out"] for i in range(NCORES)]
    return np.concatenate(outs, axis=0).reshape(B, H, S, D)



# revision 8
# speedup vs baseline: 1.1533x; 1.1533x over previous
"""Trainium2 Bass kernel: Based linear attention (poly feature map, causal, normalized).

Full inputs q,k,v: [1, 16, 4096, 16] fp32. Output: [1, 16, 4096, 16] fp32.
Sharding: 16 heads over 8 cores (2 heads/core); each head is independent.

Algorithm (per head): chunked quadratic-state linear attention, C=128.
  P = 1 + s + 0.5 s^2 with s = u.k, u = q/sqrt(D).
  Intra chunk: stp[j,i] = [1|k_j].[1|u_i] = 1+s ; P = Square(stp/sqrt2) masked
  (j<=i) plus 0.5-masked (trih matmul). Cross chunk, explicit quadratic
  features: q2[i,(p,r)] = 0.5 u_p u_r (PE-transposed to [f,i]),
  k2[j,(p,r)] = k_p k_r; states M2[f,d'] = sum k2^T v', M1[e,d'] = sum
  [1|k]^T v' accumulate in PSUM; numerator = intra matmuls + q2t^T @ M2 +
  [1|u]^T-read of M1, all accumulated in one PSUM tile (v' = [v|1] carries
  the normalizer z in channel 16). Normalization batched at the end.
  k/a transposed feature tiles come from DMA xbar transposes.
"""
import numpy as np
from contextlib import ExitStack

import concourse.bass as bass
import concourse.bacc as bacc
import concourse.tile as tile
import concourse.mybir as mybir
from bass_rust import add_dep_helper
from concourse.masks import make_upper_triangular
from concourse.bass_utils import run_bass_kernel_spmd

B, H, S, D = 1, 16, 4096, 16
NCORES = 8
HPC = H // NCORES  # heads per core (2)
C = 128            # chunk (positions)
NCH = S // C       # 32 chunks
D1 = D + 1         # 17
F2 = 256           # quadratic features (p,r)
FP = 32            # padded feature width for xbar transpose (NCH*FP % 128 == 0, FP % 16 == 0)
dt = mybir.dt.float32
bt = mybir.dt.bfloat16
SCALE = 1.0 / np.sqrt(D)
RT2I = 1.0 / np.sqrt(2.0)
Alu = mybir.AluOpType
Act = mybir.ActivationFunctionType


def _ap(base_ap, offset_ap, dims):
    """AP on the same tensor as `base_ap`: partition dim kept, free dims replaced."""
    return bass.AP(tensor=base_ap.tensor, offset=offset_ap.offset,
                   ap=[base_ap.ap[0]] + dims)


def _build_core(nc, pools, q_d, k_d, v_d, o_d):
    (mask_ident, trih), bulk, sb, snapp, ps_stp, ps_q2t, ps_num, ps_state = pools
    ident = mask_ident  # scores/transpose identity is separate below

    # ---- raw loads (both heads): [h, S, D] -> [128, h, NCH, D] ----
    qraw = bulk.tile([128, HPC, NCH, D], dt, tag="qraw")
    kraw = bulk.tile([128, HPC, NCH, D], dt, tag="kraw")
    vraw = bulk.tile([128, HPC, NCH, D], dt, tag="vraw")
    for h in range(HPC):
        nc.sync.dma_start(kraw[:, h], k_d[h].rearrange("(c p) d -> p c d", p=128))
        nc.sync.dma_start(qraw[:, h], q_d[h].rearrange("(c p) d -> p c d", p=128))
        nc.sync.dma_start(vraw[:, h], v_d[h].rearrange("(c p) d -> p c d", p=128))

    # ---- feature tensors ----
    # kb = [1|k] (17 used, padded to 20 for the DMA xbar transpose), ab = [1|u]
    kb = bulk.tile([128, HPC, NCH, FP], bt, tag="kb")
    ab = bulk.tile([128, HPC, NCH, FP], bt, tag="ab")
    vb = bulk.tile([128, HPC, NCH, D1], bt, tag="vb")
    nc.vector.memset(kb[:, :, :, 0:1], 1.0)
    nc.vector.memset(ab[:, :, :, 0:1], 1.0)
    nc.vector.memset(vb[:, :, :, D : D + 1], 1.0)
    nc.scalar.copy(kb[:, :, :, 1 : D + 1], kraw[:])
    nc.scalar.mul(ab[:, :, :, 1 : D + 1], qraw[:], SCALE)
    nc.gpsimd.tensor_copy(vb[:, :, :, 0:D], vraw[:])

    # ---- transposed feature tiles via DMA xbar ----
    # real xbar layout: out[r, g, j] = in[j, 128*g + r]; with FP=32 each
    # 128-col group g packs 4 chunks at partition bases {0,32,64,96}.
    ktp = bulk.tile([128, HPC, NCH // 4, 128], bt, tag="ktp")
    atp = bulk.tile([128, HPC, NCH // 4, 128], bt, tag="atp")
    nc.vector.memset(kb[:, :, :, D1:FP], 0.0)
    nc.vector.memset(ab[:, :, :, D1:FP], 0.0)
    for h in range(HPC):
        nc.sync.dma_start_transpose(ktp[:, h], kb[:, h])
        nc.sync.dma_start_transpose(atp[:, h], ab[:, h])

    tot = bulk.tile([128, HPC, NCH, D1], dt, tag="tot")
    o_sb = bulk.tile([128, HPC, NCH, D], dt, tag="osb")

    # persistent PSUM state: [128, h, 3, 17] = (M2a, M2b, M1)
    st = ps_state.tile([128, HPC, 3, D1], dt, tag="st")
    nc.vector.memset(st[:], 0.0)

    # per-chunk rotating tiles
    def q2_build(dst, src, h, scalar):
        # dst[i, f=(p*16+r)] = scalar * src[i,1+p] * src[i,1+r]
        in0 = _ap(src[:], src[:, h, 0, 1:2], [[1, D], [0, D]])
        in1 = _ap(src[:], src[:, h, 0, 1:2], [[0, D], [1, D]])
        return in0, in1

    snap_prev = None
    prev_snap_op = None
    q2t_sb_prev = None
    pt_prev = None
    cdata = {}

    # software pipeline: at iteration c, issue front-end for chunk c
    # (builds, transposes, scores, square, mask, q2t copy) and back-end for
    # chunk c-1 (readout matmuls, state update, snapshot, tot copy).
    for c in range(NCH + 1):
        if c < NCH:
            # --- DVE: quadratic feature builds ---
            q2 = sb.tile([128, HPC, F2], bt, tag="q2")
            k2 = sb.tile([128, HPC, F2], bt, tag="k2")
            for h in range(HPC):
                a0 = _ap(ab[:], ab[:, h, c, 1:2], [[1, D], [0, D]])
                a1 = _ap(ab[:], ab[:, h, c, 1:2], [[0, D], [1, D]])
                nc.vector.scalar_tensor_tensor(q2[:, h], a0, 0.5, a1,
                                               Alu.mult, Alu.mult)
                k0 = _ap(kb[:], kb[:, h, c, 1:2], [[1, D], [0, D]])
                k1 = _ap(kb[:], kb[:, h, c, 1:2], [[0, D], [1, D]])
                nc.vector.scalar_tensor_tensor(k2[:, h], k0, 1.0, k1,
                                               Alu.mult, Alu.mult)

            # --- PE: q2 transposes (via identity) -> PSUM fp32 ---
            q2t_ps = ps_q2t.tile([128, HPC, 2, 128], dt, tag="q2t")
            for h in range(HPC):
                for t in range(2):
                    nc.tensor.matmul(q2t_ps[:, h, t, :],
                                     q2[:, h, 128 * t : 128 * t + 128],
                                     ident[:], start=True, stop=True,
                                     skip_group_check=True)

            # --- PE: intra scores ---
            stp = ps_stp.tile([128, HPC, 128], dt, tag="stp")
            g, p0 = c // 4, 32 * (c % 4)
            for h in range(HPC):
                nc.tensor.matmul(stp[:, h, :], ktp[p0 : p0 + D1, h, g, :],
                                 atp[p0 : p0 + D1, h, g, :],
                                 start=True, stop=True, skip_group_check=True,
                                 tile_position=(p0, 0))

            # --- Act: q2t copy PSUM->SBUF; Square ---
            q2t_sb = sb.tile([128, HPC, 2, 128], bt, tag="q2tsb")
            nc.scalar.copy(q2t_sb[:], q2t_ps[:])
            sq = sb.tile([128, HPC, 128], bt, tag="sq")
            nc.scalar.activation(sq[:], stp[:], Act.Square, scale=RT2I)

            # --- Pool: causal mask (keep j <= i) ---
            pt = sb.tile([128, HPC, 128], bt, tag="pt")
            nc.gpsimd.affine_select(pt[:], sq[:], [[0, HPC], [1, 128]],
                                    Alu.is_ge, 0.0, base=0,
                                    channel_multiplier=-1)
            cdata[c] = (q2, k2, q2t_sb, pt)

        b = c - 1
        if b >= 0:
            q2_b, k2_b, q2t_b, pt_b = cdata.pop(b)
            # --- PE: readout matmuls -> num PSUM ---
            num = ps_num.tile([128, HPC, D1], dt, tag="num")
            for h in range(HPC):
                mms = []
                mms.append(nc.tensor.matmul(num[:, h, :], pt_b[:, h, :],
                                            vb[:, h, b, :], start=True,
                                            stop=False))
                mms.append(nc.tensor.matmul(num[:, h, :], trih[:],
                                            vb[:, h, b, :], start=False,
                                            stop=(b == 0)))
                if b > 0:
                    mms.append(nc.tensor.matmul(num[:, h, :],
                                                q2t_b[:, h, 0, :],
                                                snap_prev[:, h, 0, :],
                                                start=False, stop=False))
                    mms.append(nc.tensor.matmul(num[:, h, :],
                                                q2t_b[:, h, 1, :],
                                                snap_prev[:, h, 1, :],
                                                start=False, stop=False))
                    gb, pb = b // 4, 32 * (b % 4)
                    mms.append(nc.tensor.matmul(num[:, h, :],
                                                atp[pb : pb + D1, h, gb, :],
                                                snap_prev[pb : pb + D1, h, 2, :],
                                                start=False, stop=True,
                                                tile_position=(pb, 0)))
                for m0, m1 in zip(mms, mms[1:]):
                    add_dep_helper(m1.ins, m0.ins, reason="num accum order")

            # --- PE: state update (after previous snapshot read) ---
            umms = []
            for h in range(HPC):
                umms.append(nc.tensor.matmul(st[:, h, 0, :],
                                             k2_b[:, h, 0:128], vb[:, h, b, :],
                                             start=False, stop=False,
                                             skip_group_check=True))
                umms.append(nc.tensor.matmul(st[:, h, 1, :],
                                             k2_b[:, h, 128:256], vb[:, h, b, :],
                                             start=False, stop=False,
                                             skip_group_check=True))
                for rb in range(4):
                    umms.append(nc.tensor.matmul(st[32 * rb : 32 * rb + D1, h, 2, :],
                                                 kb[:, h, b, 0:D1], vb[:, h, b, :],
                                                 start=False, stop=False,
                                                 skip_group_check=True,
                                                 tile_position=(0, 32 * rb)))
            if prev_snap_op is not None:
                for m in umms:
                    add_dep_helper(m.ins, prev_snap_op.ins,
                                   reason="state WAR after snapshot")

            # --- Pool: snapshot state; tot copy ---
            if b < NCH - 1:
                snap = snapp.tile([128, HPC, 3, D1], bt, tag="snap")
                cp = nc.vector.tensor_copy(snap[:], st[:])
                for m in umms:
                    add_dep_helper(cp.ins, m.ins, reason="snapshot after update")
                snap_prev = snap
                prev_snap_op = cp
            nc.vector.tensor_copy(tot[:, :, b, :], num[:])

    # ---- epilogue: batched normalization ----
    rec = bulk.tile([128, HPC, NCH, 1], dt, tag="rec")
    nc.vector.reciprocal(rec[:], tot[:, :, :, D : D + 1])
    rec_bc = _ap(rec[:], rec[:], [[NCH, HPC], [1, NCH], [0, D]])
    nc.vector.tensor_tensor(o_sb[:], tot[:, :, :, 0:D], rec_bc, op=Alu.mult)
    for h in range(HPC):
        nc.sync.dma_start(o_d[h].rearrange("(c p) d -> p c d", p=128), o_sb[:, h])


def build_program():
    nc = bacc.Bacc("TRN2", target_bir_lowering=False, debug=False)
    q_d = nc.dram_tensor("q", [HPC, S, D], dt, kind="ExternalInput")
    k_d = nc.dram_tensor("k", [HPC, S, D], dt, kind="ExternalInput")
    v_d = nc.dram_tensor("v", [HPC, S, D], dt, kind="ExternalInput")
    o_d = nc.dram_tensor("out", [HPC, S, D], dt, kind="ExternalOutput")

    with tile.TileContext(nc) as tc, ExitStack() as ctx:
        constp = ctx.enter_context(tc.tile_pool(name="const", bufs=1))
        bulk = ctx.enter_context(tc.tile_pool(name="bulk", bufs=1))
        sb = ctx.enter_context(tc.tile_pool(name="sb", bufs=3))
        snapp = ctx.enter_context(tc.tile_pool(name="snap", bufs=2))
        ps_stp = ctx.enter_context(tc.tile_pool(name="ps_stp", bufs=2, space="PSUM"))
        ps_q2t = ctx.enter_context(tc.tile_pool(name="ps_q2t", bufs=2, space="PSUM"))
        ps_num = ctx.enter_context(tc.tile_pool(name="ps_num", bufs=2, space="PSUM"))
        ps_state = ctx.enter_context(tc.tile_pool(name="ps_st", bufs=1, space="PSUM"))

        from concourse.masks import make_identity
        ident = constp.tile([128, 128], bt)
        make_identity(nc, ident)
        trih = constp.tile([128, 128], bt)
        make_upper_triangular(nc, trih, val=0.5, diag=True)

        pools = ((ident, trih), bulk, sb, snapp, ps_stp, ps_q2t, ps_num, ps_state)
        _build_core(nc, pools, q_d, k_d, v_d, o_d)

    nc.compile()
    return nc


_NC = None


def kernel(q: np.ndarray, k: np.ndarray, v: np.ndarray) -> np.ndarray:
    global _NC
    if _NC is None:
        _NC = build_program()
    q = np.ascontiguousarray(np.asarray(q, dtype=np.float32).reshape(H, S, D))
    k = np.ascontiguousarray(np.asarray(k, dtype=np.float32).reshape(H, S, D))
    v = np.ascontiguousarray(np.asarray(v, dtype=np.float32).reshape(H, S, D))
    in_maps = []
    for i in range(NCORES):
        sl = slice(i * HPC, (i + 1) * HPC)
        in_maps.append({
            "q": np.ascontiguousarray(q[sl]),
            "k": np.ascontiguousarray(k[sl]),
            "v": np.ascontiguousarray(v[sl]),
        })
    res = run_bass_kernel_spmd(_NC, in_maps, core_ids=list(range(NCORES)))
    outs = [res.results[i]["out"] for i in range(NCORES)]
    return np.concatenate(outs, axis=0).reshape(B, H, S, D)


# revision 12
# speedup vs baseline: 1.3613x; 1.1804x over previous
"""Trainium2 Bass kernel: Based linear attention (poly feature map, causal, normalized).

Full inputs q,k,v: [1, 16, 4096, 16] fp32. Output: [1, 16, 4096, 16] fp32.
Sharding: 16 heads over 8 cores (2 heads/core); each head is independent.

Algorithm (per head): chunked quadratic-state linear attention, C=128.
  P = 1 + s + 0.5 s^2 with s = u.k, u = q/sqrt(D).
  Intra chunk: stp[j,i] = [1|k_j].[1|u_i] = 1+s ; P = Square(stp/sqrt2) masked
  (j<=i) plus 0.5-masked (trih matmul). Cross chunk, explicit quadratic
  features: q2[i,(p,r)] = 0.5 u_p u_r (PE-transposed to [f,i]),
  k2[j,(p,r)] = k_p k_r; states M2[f,d'] = sum k2^T v', M1[e,d'] = sum
  [1|k]^T v' accumulate in PSUM; numerator = intra matmuls + q2t^T @ M2 +
  [1|u]^T-read of M1, all accumulated in one PSUM tile (v' = [v|1] carries
  the normalizer z in channel 16). Normalization batched at the end.
  k/a transposed feature tiles come from DMA xbar transposes.
"""
import numpy as np
from contextlib import ExitStack

import concourse.bass as bass
import concourse.bacc as bacc
import concourse.tile as tile
import concourse.mybir as mybir
from bass_rust import add_dep_helper
from concourse.masks import make_upper_triangular
from concourse.bass_utils import run_bass_kernel_spmd

B, H, S, D = 1, 16, 4096, 16
NCORES = 8
HPC = H // NCORES  # heads per core (2)
C = 128            # chunk (positions)
NCH = S // C       # 32 chunks
D1 = D + 1         # 17
F2 = 256           # quadratic features (p,r)
FP = 32            # padded feature width for xbar transpose (NCH*FP % 128 == 0, FP % 16 == 0)
dt = mybir.dt.float32
bt = mybir.dt.bfloat16
SCALE = 1.0 / np.sqrt(D)
RT2I = 1.0 / np.sqrt(2.0)
Alu = mybir.AluOpType
Act = mybir.ActivationFunctionType


def _ap(base_ap, offset_ap, dims):
    """AP on the same tensor as `base_ap`: partition dim kept, free dims replaced."""
    return bass.AP(tensor=base_ap.tensor, offset=offset_ap.offset,
                   ap=[base_ap.ap[0]] + dims)


def _build_core(nc, pools, q_d, k_d, v_d, o_d):
    (mask_ident, trih), bulk, sb, snapp, ps_stp, ps_q2t, ps_num, ps_state = pools
    ident = mask_ident  # scores/transpose identity is separate below

    # ---- raw loads (both heads): [h, S, D] -> [128, h, NCH, D] ----
    qraw = bulk.tile([128, HPC, NCH, D], dt, tag="qraw")
    kraw = bulk.tile([128, HPC, NCH, D], dt, tag="kraw")
    vraw = bulk.tile([128, HPC, NCH, D], dt, tag="vraw")
    for h in range(HPC):
        nc.sync.dma_start(kraw[:, h], k_d[h].rearrange("(c p) d -> p c d", p=128))
        nc.sync.dma_start(qraw[:, h], q_d[h].rearrange("(c p) d -> p c d", p=128))
        nc.sync.dma_start(vraw[:, h], v_d[h].rearrange("(c p) d -> p c d", p=128))

    # ---- feature tensors ----
    # kb = [1|k] (17 used, padded to 20 for the DMA xbar transpose), ab = [1|u]
    kb = bulk.tile([128, HPC, NCH, FP], bt, tag="kb")
    ab = bulk.tile([128, HPC, NCH, FP], bt, tag="ab")
    vb = bulk.tile([128, HPC, NCH, D1], bt, tag="vb")
    nc.vector.memset(kb[:, :, :, 0:1], 1.0)
    nc.vector.memset(ab[:, :, :, 0:1], 1.0)
    nc.vector.memset(vb[:, :, :, D : D + 1], 1.0)
    nc.scalar.copy(kb[:, :, :, 1 : D + 1], kraw[:])
    nc.scalar.mul(ab[:, :, :, 1 : D + 1], qraw[:], SCALE)
    nc.gpsimd.tensor_copy(vb[:, :, :, 0:D], vraw[:])

    # ---- transposed feature tiles via DMA xbar ----
    # real xbar layout: out[r, g, j] = in[j, 128*g + r]; with FP=32 each
    # 128-col group g packs 4 chunks at partition bases {0,32,64,96}.
    ktp = bulk.tile([128, HPC, NCH // 4, 128], bt, tag="ktp")
    atp = bulk.tile([128, HPC, NCH // 4, 128], bt, tag="atp")
    nc.vector.memset(kb[:, :, :, D1:FP], 0.0)
    nc.vector.memset(ab[:, :, :, D1:FP], 0.0)
    for h in range(HPC):
        nc.sync.dma_start_transpose(ktp[:, h], kb[:, h])
        nc.sync.dma_start_transpose(atp[:, h], ab[:, h])

    tot = bulk.tile([128, HPC, NCH, D1], dt, tag="tot")
    o_sb = bulk.tile([128, HPC, NCH, D], dt, tag="osb")

    # persistent PSUM state: [128, h, 3, 17] = (M2a, M2b, M1)
    st = ps_state.tile([128, HPC, 3, D1], dt, tag="st")
    nc.vector.memset(st[:], 0.0)

    # per-chunk rotating tiles
    def q2_build(dst, src, h, scalar):
        # dst[i, f=(p*16+r)] = scalar * src[i,1+p] * src[i,1+r]
        in0 = _ap(src[:], src[:, h, 0, 1:2], [[1, D], [0, D]])
        in1 = _ap(src[:], src[:, h, 0, 1:2], [[0, D], [1, D]])
        return in0, in1

    snap_prev = None
    prev_snap_op = None
    q2t_sb_prev = None
    pt_prev = None
    cdata = {}

    # software pipeline: at iteration c, issue front-end for chunk c
    # (builds, transposes, scores, square, mask, q2t copy) and back-end for
    # chunk c-1 (readout matmuls, state update, snapshot, tot copy).
    for c in range(NCH + 1):
        if c < NCH:
            # --- DVE: quadratic feature builds ---
            q2 = sb.tile([128, HPC, F2], bt, tag="q2")
            k2 = sb.tile([128, HPC, F2], bt, tag="k2")
            for h in range(HPC):
                a0 = _ap(ab[:], ab[:, h, c, 1:2], [[1, D], [0, D]])
                a1 = _ap(ab[:], ab[:, h, c, 1:2], [[0, D], [1, D]])
                nc.vector.scalar_tensor_tensor(q2[:, h], a0, 0.5, a1,
                                               Alu.mult, Alu.mult)
            k00 = _ap(kb[:], kb[:, 0, c, 1:2], [[1, D], [0, D]])
            k01 = _ap(kb[:], kb[:, 0, c, 1:2], [[0, D], [1, D]])
            nc.gpsimd.tensor_mul(k2[:, 0], k00, k01)
            k10 = _ap(kb[:], kb[:, 1, c, 1:2], [[1, D], [0, D]])
            k11 = _ap(kb[:], kb[:, 1, c, 1:2], [[0, D], [1, D]])
            nc.vector.scalar_tensor_tensor(k2[:, 1], k10, 1.0, k11,
                                           Alu.mult, Alu.mult)

            # --- PE: q2 transposes (via identity) -> PSUM fp32 ---
            q2t_ps = ps_q2t.tile([128, HPC, 2, 128], dt, tag="q2t")
            for h in range(HPC):
                for t in range(2):
                    nc.tensor.matmul(q2t_ps[:, h, t, :],
                                     q2[:, h, 128 * t : 128 * t + 128],
                                     ident[:], start=True, stop=True,
                                     skip_group_check=True)

            # --- PE: intra scores ---
            stp = ps_stp.tile([128, HPC, 128], dt, tag="stp")
            g, p0 = c // 4, 32 * (c % 4)
            for h in range(HPC):
                nc.tensor.matmul(stp[:, h, :], ktp[p0 : p0 + D1, h, g, :],
                                 atp[p0 : p0 + D1, h, g, :],
                                 start=True, stop=True, skip_group_check=True,
                                 tile_position=(p0, 0))

            # --- Act: q2t copy PSUM->SBUF; Square ---
            q2t_sb = sb.tile([128, HPC, 2, 128], bt, tag="q2tsb")
            nc.scalar.copy(q2t_sb[:], q2t_ps[:])
            sq = sb.tile([128, HPC, 128], bt, tag="sq")
            nc.scalar.activation(sq[:], stp[:], Act.Square, scale=RT2I)

            # --- Pool: causal mask (keep j <= i) ---
            pt = sb.tile([128, HPC, 128], bt, tag="pt")
            nc.gpsimd.affine_select(pt[:], sq[:], [[0, HPC], [1, 128]],
                                    Alu.is_ge, 0.0, base=0,
                                    channel_multiplier=-1)
            cdata[c] = (q2, k2, q2t_sb, pt)

        b = c - 1
        if b >= 0:
            q2_b, k2_b, q2t_b, pt_b = cdata.pop(b)
            # --- PE: readout matmuls -> num PSUM ---
            num = ps_num.tile([128, HPC, D1], dt, tag="num")
            for h in range(HPC):
                mms = []
                mms.append(nc.tensor.matmul(num[:, h, :], pt_b[:, h, :],
                                            vb[:, h, b, :], start=True,
                                            stop=False))
                mms.append(nc.tensor.matmul(num[:, h, :], trih[:],
                                            vb[:, h, b, :], start=False,
                                            stop=(b == 0)))
                if b > 0:
                    mms.append(nc.tensor.matmul(num[:, h, :],
                                                q2t_b[:, h, 0, :],
                                                snap_prev[:, h, 0, :],
                                                start=False, stop=False))
                    mms.append(nc.tensor.matmul(num[:, h, :],
                                                q2t_b[:, h, 1, :],
                                                snap_prev[:, h, 1, :],
                                                start=False, stop=False))
                    gb, pb = b // 4, 32 * (b % 4)
                    mms.append(nc.tensor.matmul(num[:, h, :],
                                                atp[pb : pb + D1, h, gb, :],
                                                snap_prev[pb : pb + D1, h, 2, :],
                                                start=False, stop=True,
                                                tile_position=(pb, 0)))
                for m0, m1 in zip(mms, mms[1:]):
                    add_dep_helper(m1.ins, m0.ins, reason="num accum order")

            # --- PE: state update (after previous snapshot read) ---
            umms = []
            for h in range(HPC):
                umms.append(nc.tensor.matmul(st[:, h, 0, :],
                                             k2_b[:, h, 0:128], vb[:, h, b, :],
                                             start=False, stop=False,
                                             skip_group_check=True))
                umms.append(nc.tensor.matmul(st[:, h, 1, :],
                                             k2_b[:, h, 128:256], vb[:, h, b, :],
                                             start=False, stop=False,
                                             skip_group_check=True))
                for rb in range(4):
                    umms.append(nc.tensor.matmul(st[32 * rb : 32 * rb + D1, h, 2, :],
                                                 kb[:, h, b, 0:D1], vb[:, h, b, :],
                                                 start=False, stop=False,
                                                 skip_group_check=True,
                                                 tile_position=(0, 32 * rb)))
            if prev_snap_op is not None:
                for m in umms:
                    add_dep_helper(m.ins, prev_snap_op.ins,
                                   reason="state WAR after snapshot")

            # --- Pool: snapshot state; tot copy ---
            if b < NCH - 1:
                snap = snapp.tile([128, HPC, 3, D1], bt, tag="snap")
                cp = nc.scalar.copy(snap[:], st[:])
                for m in umms:
                    add_dep_helper(cp.ins, m.ins, reason="snapshot after update")
                snap_prev = snap
                prev_snap_op = cp
            nc.vector.tensor_copy(tot[:, :, b, :], num[:])

    # ---- epilogue: batched normalization ----
    rec = bulk.tile([128, HPC, NCH, 1], dt, tag="rec")
    nc.vector.reciprocal(rec[:], tot[:, :, :, D : D + 1])
    rec_bc = _ap(rec[:], rec[:], [[NCH, HPC], [1, NCH], [0, D]])
    nc.vector.tensor_tensor(o_sb[:], tot[:, :, :, 0:D], rec_bc, op=Alu.mult)
    for h in range(HPC):
        nc.sync.dma_start(o_d[h].rearrange("(c p) d -> p c d", p=128), o_sb[:, h])


def build_program():
    nc = bacc.Bacc("TRN2", target_bir_lowering=False, debug=False)
    q_d = nc.dram_tensor("q", [HPC, S, D], dt, kind="ExternalInput")
    k_d = nc.dram_tensor("k", [HPC, S, D], dt, kind="ExternalInput")
    v_d = nc.dram_tensor("v", [HPC, S, D], dt, kind="ExternalInput")
    o_d = nc.dram_tensor("out", [HPC, S, D], dt, kind="ExternalOutput")

    with tile.TileContext(nc) as tc, ExitStack() as ctx:
        constp = ctx.enter_context(tc.tile_pool(name="const", bufs=1))
        bulk = ctx.enter_context(tc.tile_pool(name="bulk", bufs=1))
        sb = ctx.enter_context(tc.tile_pool(name="sb", bufs=3))
        snapp = ctx.enter_context(tc.tile_pool(name="snap", bufs=2))
        ps_stp = ctx.enter_context(tc.tile_pool(name="ps_stp", bufs=2, space="PSUM"))
        ps_q2t = ctx.enter_context(tc.tile_pool(name="ps_q2t", bufs=2, space="PSUM"))
        ps_num = ctx.enter_context(tc.tile_pool(name="ps_num", bufs=2, space="PSUM"))
        ps_state = ctx.enter_context(tc.tile_pool(name="ps_st", bufs=1, space="PSUM"))

        from concourse.masks import make_identity
        ident = constp.tile([128, 128], bt)
        make_identity(nc, ident)
        trih = constp.tile([128, 128], bt)
        make_upper_triangular(nc, trih, val=0.5, diag=True)

        pools = ((ident, trih), bulk, sb, snapp, ps_stp, ps_q2t, ps_num, ps_state)
        _build_core(nc, pools, q_d, k_d, v_d, o_d)

    nc.compile()
    return nc


_NC = None


def kernel(q: np.ndarray, k: np.ndarray, v: np.ndarray) -> np.ndarray:
    global _NC
    if _NC is None:
        _NC = build_program()
    q = np.ascontiguousarray(np.asarray(q, dtype=np.float32).reshape(H, S, D))
    k = np.ascontiguousarray(np.asarray(k, dtype=np.float32).reshape(H, S, D))
    v = np.ascontiguousarray(np.asarray(v, dtype=np.float32).reshape(H, S, D))
    in_maps = []
    for i in range(NCORES):
        sl = slice(i * HPC, (i + 1) * HPC)
        in_maps.append({
            "q": np.ascontiguousarray(q[sl]),
            "k": np.ascontiguousarray(k[sl]),
            "v": np.ascontiguousarray(v[sl]),
        })
    res = run_bass_kernel_spmd(_NC, in_maps, core_ids=list(range(NCORES)))
    outs = [res.results[i]["out"] for i in range(NCORES)]
    return np.concatenate(outs, axis=0).reshape(B, H, S, D)


# revision 13
# speedup vs baseline: 1.4616x; 1.0736x over previous
"""Trainium2 Bass kernel: Based linear attention (poly feature map, causal, normalized).

Full inputs q,k,v: [1, 16, 4096, 16] fp32. Output: [1, 16, 4096, 16] fp32.
Sharding: 16 heads over 8 cores (2 heads/core); each head is independent.

Algorithm (per head): chunked quadratic-state linear attention, C=128.
  P = 1 + s + 0.5 s^2 with s = u.k, u = q/sqrt(D).
  Intra chunk: stp[j,i] = [1|k_j].[1|u_i] = 1+s ; P = Square(stp/sqrt2) masked
  (j<=i) plus 0.5-masked (trih matmul). Cross chunk, explicit quadratic
  features: q2[i,(p,r)] = 0.5 u_p u_r (PE-transposed to [f,i]),
  k2[j,(p,r)] = k_p k_r; states M2[f,d'] = sum k2^T v', M1[e,d'] = sum
  [1|k]^T v' accumulate in PSUM; numerator = intra matmuls + q2t^T @ M2 +
  [1|u]^T-read of M1, all accumulated in one PSUM tile (v' = [v|1] carries
  the normalizer z in channel 16). Normalization batched at the end.
  k/a transposed feature tiles come from DMA xbar transposes.
"""
import numpy as np
from contextlib import ExitStack

import concourse.bass as bass
import concourse.bacc as bacc
import concourse.tile as tile
import concourse.mybir as mybir
from bass_rust import add_dep_helper
from concourse.masks import make_upper_triangular
from concourse.bass_utils import run_bass_kernel_spmd

B, H, S, D = 1, 16, 4096, 16
NCORES = 8
HPC = H // NCORES  # heads per core (2)
C = 128            # chunk (positions)
NCH = S // C       # 32 chunks
D1 = D + 1         # 17
F2 = 256           # quadratic features (p,r)
FP = 32            # padded feature width for xbar transpose (NCH*FP % 128 == 0, FP % 16 == 0)
dt = mybir.dt.float32
bt = mybir.dt.bfloat16
SCALE = 1.0 / np.sqrt(D)
RT2I = 1.0 / np.sqrt(2.0)
Alu = mybir.AluOpType
Act = mybir.ActivationFunctionType


def _ap(base_ap, offset_ap, dims):
    """AP on the same tensor as `base_ap`: partition dim kept, free dims replaced."""
    return bass.AP(tensor=base_ap.tensor, offset=offset_ap.offset,
                   ap=[base_ap.ap[0]] + dims)


def _build_core(nc, pools, q_d, k_d, v_d, o_d):
    (ident, trih, mask), bulk, sb, snapp, ps_stp, ps_q2t, ps_num, ps_state = pools

    # ---- raw loads (both heads): [h, S, D] -> [128, h, NCH, D] ----
    qraw = bulk.tile([128, HPC, NCH, D], dt, tag="qraw")
    kraw = bulk.tile([128, HPC, NCH, D], dt, tag="kraw")
    vraw = bulk.tile([128, HPC, NCH, D], dt, tag="vraw")
    nc.sync.dma_start(kraw[:], k_d.rearrange("h (c p) d -> p h c d", p=128))
    nc.sync.dma_start(qraw[:], q_d.rearrange("h (c p) d -> p h c d", p=128))
    nc.sync.dma_start(vraw[:], v_d.rearrange("h (c p) d -> p h c d", p=128))

    # ---- feature tensors ----
    # kb = [1|k] (17 used, padded to 20 for the DMA xbar transpose), ab = [1|u]
    kb = bulk.tile([128, HPC, NCH, FP], bt, tag="kb")
    ab = bulk.tile([128, HPC, NCH, FP], bt, tag="ab")
    vb = bulk.tile([128, HPC, NCH, D1], bt, tag="vb")
    nc.vector.memset(kb[:, :, :, 0:1], 1.0)
    nc.vector.memset(ab[:, :, :, 0:1], 1.0)
    nc.vector.memset(vb[:, :, :, D : D + 1], 1.0)
    nc.scalar.copy(kb[:, :, :, 1 : D + 1], kraw[:])
    nc.scalar.mul(ab[:, :, :, 1 : D + 1], qraw[:], SCALE)
    nc.gpsimd.tensor_copy(vb[:, :, :, 0:D], vraw[:])

    # ---- transposed feature tiles via DMA xbar ----
    # real xbar layout: out[r, g, j] = in[j, 128*g + r]; with FP=32 each
    # 128-col group g packs 4 chunks at partition bases {0,32,64,96}.
    ktp = bulk.tile([128, HPC, NCH // 4, 128], bt, tag="ktp")
    atp = bulk.tile([128, HPC, NCH // 4, 128], bt, tag="atp")
    nc.vector.memset(kb[:, :, :, D1:FP], 0.0)
    nc.vector.memset(ab[:, :, :, D1:FP], 0.0)
    for h in range(HPC):
        nc.sync.dma_start_transpose(ktp[:, h], kb[:, h])
        nc.sync.dma_start_transpose(atp[:, h], ab[:, h])

    tot = bulk.tile([128, HPC, NCH, D1], dt, tag="tot")
    o_sb = bulk.tile([128, HPC, NCH, D], dt, tag="osb")

    # persistent PSUM state: [128, h, 3, 17] = (M2a, M2b, M1)
    st = ps_state.tile([128, HPC, 3, D1], dt, tag="st")
    nc.vector.memset(st[:], 0.0)

    # per-chunk rotating tiles
    def q2_build(dst, src, h, scalar):
        # dst[i, f=(p*16+r)] = scalar * src[i,1+p] * src[i,1+r]
        in0 = _ap(src[:], src[:, h, 0, 1:2], [[1, D], [0, D]])
        in1 = _ap(src[:], src[:, h, 0, 1:2], [[0, D], [1, D]])
        return in0, in1

    snap_prev = None
    prev_snap_op = None
    q2t_sb_prev = None
    pt_prev = None
    cdata = {}

    # software pipeline: at iteration c, issue front-end for chunk c
    # (builds, transposes, scores, square, mask, q2t copy) and back-end for
    # chunk c-1 (readout matmuls, state update, snapshot, tot copy).
    for c in range(NCH + 1):
        if c < NCH:
            # --- DVE: quadratic feature builds ---
            q2 = sb.tile([128, HPC, F2], bt, tag="q2")
            k2 = sb.tile([128, HPC, F2], bt, tag="k2")
            for h in range(HPC):
                a0 = _ap(ab[:], ab[:, h, c, 1:2], [[1, D], [0, D]])
                a1 = _ap(ab[:], ab[:, h, c, 1:2], [[0, D], [1, D]])
                nc.vector.scalar_tensor_tensor(q2[:, h], a0, 1.0, a1,
                                               Alu.mult, Alu.mult)
            k00 = _ap(kb[:], kb[:, 0, c, 1:2], [[1, D], [0, D]])
            k01 = _ap(kb[:], kb[:, 0, c, 1:2], [[0, D], [1, D]])
            nc.gpsimd.tensor_mul(k2[:, 0], k00, k01)
            k10 = _ap(kb[:], kb[:, 1, c, 1:2], [[1, D], [0, D]])
            k11 = _ap(kb[:], kb[:, 1, c, 1:2], [[0, D], [1, D]])
            nc.gpsimd.tensor_mul(k2[:, 1], k10, k11)

            # --- PE: q2 transposes (via identity) -> PSUM fp32 ---
            q2t_ps = ps_q2t.tile([128, HPC, 2, 128], bt, tag="q2t")
            for h in range(HPC):
                for t in range(2):
                    nc.tensor.matmul(q2t_ps[:, h, t, :],
                                     q2[:, h, 128 * t : 128 * t + 128],
                                     ident[:], start=True, stop=True,
                                     skip_group_check=True, is_transpose=True)

            # --- PE: intra scores ---
            stp = ps_stp.tile([128, HPC, 128], dt, tag="stp")
            g, p0 = c // 4, 32 * (c % 4)
            for h in range(HPC):
                nc.tensor.matmul(stp[:, h, :], ktp[p0 : p0 + D1, h, g, :],
                                 atp[p0 : p0 + D1, h, g, :],
                                 start=True, stop=True, skip_group_check=True,
                                 tile_position=(p0, 0))

            # --- Act: Square; DVE: q2t copy (x0.5) and causal mask ---
            sq = sb.tile([128, HPC, 128], bt, tag="sq")
            nc.scalar.activation(sq[:], stp[:], Act.Square, scale=RT2I)
            q2t_sb = sb.tile([128, HPC, 2, 128], bt, tag="q2tsb")
            nc.vector.tensor_scalar_mul(q2t_sb[:], q2t_ps[:], 0.5)
            pt = sb.tile([128, HPC, 128], bt, tag="pt")
            mask_bc = _ap(mask[:], mask[:], [[0, HPC], [1, 128]])
            nc.vector.tensor_mul(pt[:], sq[:], mask_bc)
            cdata[c] = (q2, k2, q2t_sb, pt)

        b = c - 1
        if b >= 0:
            q2_b, k2_b, q2t_b, pt_b = cdata.pop(b)
            # --- PE: readout matmuls -> num PSUM ---
            num = ps_num.tile([128, HPC, D1], dt, tag="num")
            for h in range(HPC):
                mms = []
                mms.append(nc.tensor.matmul(num[:, h, :], pt_b[:, h, :],
                                            vb[:, h, b, :], start=True,
                                            stop=False))
                mms.append(nc.tensor.matmul(num[:, h, :], trih[:],
                                            vb[:, h, b, :], start=False,
                                            stop=(b == 0)))
                if b > 0:
                    mms.append(nc.tensor.matmul(num[:, h, :],
                                                q2t_b[:, h, 0, :],
                                                snap_prev[:, h, 0, :],
                                                start=False, stop=False))
                    mms.append(nc.tensor.matmul(num[:, h, :],
                                                q2t_b[:, h, 1, :],
                                                snap_prev[:, h, 1, :],
                                                start=False, stop=False))
                    gb, pb = b // 4, 32 * (b % 4)
                    mms.append(nc.tensor.matmul(num[:, h, :],
                                                atp[pb : pb + D1, h, gb, :],
                                                snap_prev[pb : pb + D1, h, 2, :],
                                                start=False, stop=True,
                                                tile_position=(pb, 0)))
                for m0, m1 in zip(mms, mms[1:]):
                    add_dep_helper(m1.ins, m0.ins, reason="num accum order")

            # --- PE: state update (after previous snapshot read) ---
            umms = []
            for h in range(HPC):
                umms.append(nc.tensor.matmul(st[:, h, 0, :],
                                             k2_b[:, h, 0:128], vb[:, h, b, :],
                                             start=False, stop=False,
                                             skip_group_check=True))
                umms.append(nc.tensor.matmul(st[:, h, 1, :],
                                             k2_b[:, h, 128:256], vb[:, h, b, :],
                                             start=False, stop=False,
                                             skip_group_check=True))
                for rb in range(4):
                    umms.append(nc.tensor.matmul(st[32 * rb : 32 * rb + D1, h, 2, :],
                                                 kb[:, h, b, 0:D1], vb[:, h, b, :],
                                                 start=False, stop=False,
                                                 skip_group_check=True,
                                                 tile_position=(0, 32 * rb)))
            if prev_snap_op is not None:
                for m in umms:
                    add_dep_helper(m.ins, prev_snap_op.ins,
                                   reason="state WAR after snapshot")

            # --- Pool: snapshot state; tot copy ---
            if b < NCH - 1:
                snap = snapp.tile([128, HPC, 3, D1], bt, tag="snap")
                cp = nc.scalar.copy(snap[:], st[:])
                for m in umms:
                    add_dep_helper(cp.ins, m.ins, reason="snapshot after update")
                snap_prev = snap
                prev_snap_op = cp
            nc.scalar.copy(tot[:, :, b, :], num[:])

    # ---- epilogue: batched normalization ----
    rec = bulk.tile([128, HPC, NCH, 1], dt, tag="rec")
    nc.vector.reciprocal(rec[:], tot[:, :, :, D : D + 1])
    rec_bc = _ap(rec[:], rec[:], [[NCH, HPC], [1, NCH], [0, D]])
    nc.vector.tensor_tensor(o_sb[:], tot[:, :, :, 0:D], rec_bc, op=Alu.mult)
    for h in range(HPC):
        nc.sync.dma_start(o_d[h].rearrange("(c p) d -> p c d", p=128), o_sb[:, h])


def build_program():
    nc = bacc.Bacc("TRN2", target_bir_lowering=False, debug=False)
    q_d = nc.dram_tensor("q", [HPC, S, D], dt, kind="ExternalInput")
    k_d = nc.dram_tensor("k", [HPC, S, D], dt, kind="ExternalInput")
    v_d = nc.dram_tensor("v", [HPC, S, D], dt, kind="ExternalInput")
    o_d = nc.dram_tensor("out", [HPC, S, D], dt, kind="ExternalOutput")

    with tile.TileContext(nc) as tc, ExitStack() as ctx:
        constp = ctx.enter_context(tc.tile_pool(name="const", bufs=1))
        bulk = ctx.enter_context(tc.tile_pool(name="bulk", bufs=1))
        sb = ctx.enter_context(tc.tile_pool(name="sb", bufs=3))
        snapp = ctx.enter_context(tc.tile_pool(name="snap", bufs=2))
        ps_stp = ctx.enter_context(tc.tile_pool(name="ps_stp", bufs=2, space="PSUM"))
        ps_q2t = ctx.enter_context(tc.tile_pool(name="ps_q2t", bufs=2, space="PSUM"))
        ps_num = ctx.enter_context(tc.tile_pool(name="ps_num", bufs=2, space="PSUM"))
        ps_state = ctx.enter_context(tc.tile_pool(name="ps_st", bufs=1, space="PSUM"))

        from concourse.masks import make_identity
        ident = constp.tile([128, 128], bt)
        make_identity(nc, ident)
        trih = constp.tile([128, 128], bt)
        make_upper_triangular(nc, trih, val=0.5, diag=True)
        mask = constp.tile([128, 128], bt)
        make_upper_triangular(nc, mask, val=1.0, diag=True)

        pools = ((ident, trih, mask), bulk, sb, snapp, ps_stp, ps_q2t, ps_num, ps_state)
        _build_core(nc, pools, q_d, k_d, v_d, o_d)

    nc.compile()
    return nc


_NC = None


def kernel(q: np.ndarray, k: np.ndarray, v: np.ndarray) -> np.ndarray:
    global _NC
    if _NC is None:
        _NC = build_program()
    q = np.ascontiguousarray(np.asarray(q, dtype=np.float32).reshape(H, S, D))
    k = np.ascontiguousarray(np.asarray(k, dtype=np.float32).reshape(H, S, D))
    v = np.ascontiguousarray(np.asarray(v, dtype=np.float32).reshape(H, S, D))
    in_maps = []
    for i in range(NCORES):
        sl = slice(i * HPC, (i + 1) * HPC)
        in_maps.append({
            "q": np.ascontiguousarray(q[sl]),
            "k": np.ascontiguousarray(k[sl]),
            "v": np.ascontiguousarray(v[sl]),
        })
    res = run_bass_kernel_spmd(_NC, in_maps, core_ids=list(range(NCORES)))
    outs = [res.results[i]["out"] for i in range(NCORES)]
    return np.concatenate(outs, axis=0).reshape(B, H, S, D)


# revision 22
# speedup vs baseline: 1.5477x; 1.0589x over previous
"""Trainium2 Bass kernel: Based linear attention (poly feature map, causal, normalized).

Full inputs q,k,v: [1, 16, 4096, 16] fp32. Output: [1, 16, 4096, 16] fp32.
Sharding: 16 heads over 8 cores (2 heads/core); each head is independent.

Algorithm (per head): chunked quadratic-state linear attention, C=128.
  P = 1 + s + 0.5 s^2 with s = u.k, u = q/sqrt(D).
  Intra chunk: stp[j,i] = [1|k_j].[1|u_i] = 1+s ; P = Square(stp/sqrt2) masked
  (j<=i) plus 0.5-masked (trih matmul). Cross chunk, explicit quadratic
  features: q2[i,(p,r)] = 0.5 u_p u_r (PE-transposed to [f,i]),
  k2[j,(p,r)] = k_p k_r; states M2[f,d'] = sum k2^T v', M1[e,d'] = sum
  [1|k]^T v' accumulate in PSUM; numerator = intra matmuls + q2t^T @ M2 +
  [1|u]^T-read of M1, all accumulated in one PSUM tile (v' = [v|1] carries
  the normalizer z in channel 16). Normalization batched at the end.
  k/a transposed feature tiles come from DMA xbar transposes.
"""
import numpy as np
from contextlib import ExitStack

import concourse.bass as bass
import concourse.bacc as bacc
import concourse.tile as tile
import concourse.mybir as mybir
from bass_rust import add_dep_helper
from concourse.masks import make_upper_triangular
from concourse.bass_utils import run_bass_kernel_spmd

B, H, S, D = 1, 16, 4096, 16
NCORES = 8
HPC = H // NCORES  # heads per core (2)
C = 128            # chunk (positions)
NCH = S // C       # 32 chunks
D1 = D + 1         # 17
F2 = 256           # quadratic features (p,r)
FP = 32            # padded feature width for xbar transpose (NCH*FP % 128 == 0, FP % 16 == 0)
dt = mybir.dt.float32
bt = mybir.dt.bfloat16
SCALE = 1.0 / np.sqrt(D)
RT2I = 1.0 / np.sqrt(2.0)
Alu = mybir.AluOpType
Act = mybir.ActivationFunctionType


def _ap(base_ap, offset_ap, dims):
    """AP on the same tensor as `base_ap`: partition dim kept, free dims replaced."""
    return bass.AP(tensor=base_ap.tensor, offset=offset_ap.offset,
                   ap=[base_ap.ap[0]] + dims)


def _build_core(nc, pools, q_d, k_d, v_d, o_d):
    (ident, trih, mask), bulk, sb, snapp, ps_stp, ps_q2t, ps_num, ps_state = pools

    # ---- raw loads (both heads): [h, S, D] -> [128, h, NCH, D] ----
    qraw = bulk.tile([128, HPC, NCH, D], dt, tag="qraw")
    kraw = bulk.tile([128, HPC, NCH, D], dt, tag="kraw")
    vraw = bulk.tile([128, HPC, NCH, D], dt, tag="vraw")
    NQ = NCH // 2

    # loads/prep/xbar are interleaved below (per half)
    # ---- feature tensors ----
    # kb = [1|k] (17 used, padded to 20 for the DMA xbar transpose), ab = [1|u]
    kb = bulk.tile([128, HPC, NCH, FP], bt, tag="kb")
    ab = bulk.tile([128, HPC, NCH, FP], bt, tag="ab")
    vb = bulk.tile([128, HPC, NCH, D1], bt, tag="vb")
    nc.vector.memset(kb[:, :, :, 0:1], 1.0)
    nc.vector.memset(ab[:, :, :, 0:1], 1.0)
    nc.vector.memset(vb[:, :, :, D : D + 1], 1.0)
    nc.vector.memset(kb[:, :, :, D1:FP], 0.0)
    nc.vector.memset(ab[:, :, :, D1:FP], 0.0)

    # real xbar layout: out[r, g, j] = in[j, 128*g + r]; with FP=32 each
    # 128-col group g packs 4 chunks at partition bases {0,32,64,96}.
    ktp = bulk.tile([128, HPC, NCH // 4, 128], bt, tag="ktp")
    atp = bulk.tile([128, HPC, NCH // 4, 128], bt, tag="atp")
    NQ4 = NCH // 2
    NG4 = NCH // 2 // 4  # xbar groups per load half
    for qt in range(2):
        cs = slice(qt * NQ4, (qt + 1) * NQ4)
        gs = slice(qt * NG4, (qt + 1) * NG4)
        for h in range(HPC):
            k_r = k_d[h].rearrange("(c p) d -> p c d", p=128)
            q_r = q_d[h].rearrange("(c p) d -> p c d", p=128)
            nc.sync.dma_start(kraw[:, h, cs], k_r[:, cs])
            nc.sync.dma_start(qraw[:, h, cs], q_r[:, cs])
        nc.scalar.copy(kb[:, :, cs, 1 : D + 1], kraw[:, :, cs])
        nc.scalar.mul(ab[:, :, cs, 1 : D + 1], qraw[:, :, cs], SCALE)
        for h in range(HPC):
            nc.sync.dma_start_transpose(ktp[:, h, gs], kb[:, h, cs])
            nc.sync.dma_start_transpose(atp[:, h, gs], ab[:, h, cs])
        for h in range(HPC):
            v_r = v_d[h].rearrange("(c p) d -> p c d", p=128)
            nc.sync.dma_start(vraw[:, h, cs], v_r[:, cs])
        nc.scalar.copy(vb[:, :, cs, 0:D], vraw[:, :, cs])

    tot = bulk.tile([128, HPC, NCH, D1], dt, tag="tot")
    o_sb = bulk.tile([128, HPC, NCH, D], dt, tag="osb")

    # persistent PSUM state: [128, h, 3, 17] = (M2a, M2b, M1)
    st = ps_state.tile([128, HPC, 3, D1], dt, tag="st")
    nc.vector.memset(st[:], 0.0)

    # per-chunk rotating tiles
    def q2_build(dst, src, h, scalar):
        # dst[i, f=(p*16+r)] = scalar * src[i,1+p] * src[i,1+r]
        in0 = _ap(src[:], src[:, h, 0, 1:2], [[1, D], [0, D]])
        in1 = _ap(src[:], src[:, h, 0, 1:2], [[0, D], [1, D]])
        return in0, in1

    snap_prev = None
    prev_snap_op = None
    q2t_sb_prev = None
    pt_prev = None
    cdata = {}

    # software pipeline: at iteration c, issue front-end for chunk c
    # (builds, transposes, scores, square, mask, q2t copy) and back-end for
    # chunk c-1 (readout matmuls, state update, snapshot, tot copy).
    for c in range(NCH + 1):
        if c < NCH:
            # --- DVE: quadratic feature builds ---
            q2 = sb.tile([128, HPC, F2], bt, tag="q2")
            k2 = sb.tile([128, HPC, F2], bt, tag="k2")
            for h in range(HPC):
                a0 = _ap(ab[:], ab[:, h, c, 1:2], [[1, D], [0, D]])
                a1 = _ap(ab[:], ab[:, h, c, 1:2], [[0, D], [1, D]])
                nc.vector.scalar_tensor_tensor(q2[:, h], a0, 1.0, a1,
                                               Alu.mult, Alu.mult)
            k00 = _ap(kb[:], kb[:, 0, c, 1:2], [[1, D], [0, D]])
            k01 = _ap(kb[:], kb[:, 0, c, 1:2], [[0, D], [1, D]])
            nc.gpsimd.tensor_mul(k2[:, 0], k00, k01)
            k10 = _ap(kb[:], kb[:, 1, c, 1:2], [[1, D], [0, D]])
            k11 = _ap(kb[:], kb[:, 1, c, 1:2], [[0, D], [1, D]])
            nc.gpsimd.tensor_mul(k2[:, 1], k10, k11)

            # --- PE: intra scores (first: feeds Square -> mask chain) ---
            stp = ps_stp.tile([128, HPC, 128], dt, tag="stp")
            g, p0 = c // 4, 32 * (c % 4)
            for h in range(HPC):
                nc.tensor.matmul(stp[:, h, :], ktp[p0 : p0 + D1, h, g, :],
                                 atp[p0 : p0 + D1, h, g, :],
                                 start=True, stop=True, skip_group_check=True,
                                 tile_position=(p0, 0))

            # --- PE: q2 transposes -> bf16 PSUM ---
            q2t_ps = ps_q2t.tile([128, HPC, 2, 128], bt, tag="q2t")
            for h in range(HPC):
                for t in range(2):
                    nc.tensor.matmul(q2t_ps[:, h, t, :],
                                     q2[:, h, 128 * t : 128 * t + 128],
                                     ident[:], start=True, stop=True,
                                     skip_group_check=True, is_transpose=True)

            # --- Act: Square; DVE: causal mask then q2t copy (x0.5) ---
            sq = sb.tile([128, HPC, 128], bt, tag="sq")
            nc.scalar.activation(sq[:], stp[:], Act.Square, scale=RT2I)
            pt = sb.tile([128, HPC, 128], bt, tag="pt")
            mask_bc = _ap(mask[:], mask[:], [[0, HPC], [1, 128]])
            nc.vector.tensor_mul(pt[:], sq[:], mask_bc)
            q2t_sb = sb.tile([128, HPC, 2, 128], bt, tag="q2tsb")
            nc.vector.tensor_scalar_mul(q2t_sb[:], q2t_ps[:], 0.5)
            cdata[c] = (q2, k2, q2t_sb, pt)

        b = c - 1
        if b >= 0:
            q2_b, k2_b, q2t_b, pt_b = cdata.pop(b)
            # --- PE: readout matmuls -> num PSUM ---
            num = ps_num.tile([128, HPC, D1], dt, tag="num")
            for h in range(HPC):
                mms = []
                mms.append(nc.tensor.matmul(num[:, h, :], pt_b[:, h, :],
                                            vb[:, h, b, :], start=True,
                                            stop=False))
                mms.append(nc.tensor.matmul(num[:, h, :], trih[:],
                                            vb[:, h, b, :], start=False,
                                            stop=(b == 0)))
                if b > 0:
                    mms.append(nc.tensor.matmul(num[:, h, :],
                                                q2t_b[:, h, 0, :],
                                                snap_prev[:, h, 0, :],
                                                start=False, stop=False))
                    mms.append(nc.tensor.matmul(num[:, h, :],
                                                q2t_b[:, h, 1, :],
                                                snap_prev[:, h, 1, :],
                                                start=False, stop=False))
                    gb, pb = b // 4, 32 * (b % 4)
                    mms.append(nc.tensor.matmul(num[:, h, :],
                                                atp[pb : pb + D1, h, gb, :],
                                                snap_prev[pb : pb + D1, h, 2, :],
                                                start=False, stop=True,
                                                tile_position=(pb, 0)))
                for m0, m1 in zip(mms, mms[1:]):
                    add_dep_helper(m1.ins, m0.ins, reason="num accum order")

            # --- PE: state update (after previous snapshot read) ---
            umms = []
            for h in range(HPC) if b < NCH - 1 else []:
                umms.append(nc.tensor.matmul(st[:, h, 0, :],
                                             k2_b[:, h, 0:128], vb[:, h, b, :],
                                             start=False, stop=False,
                                             skip_group_check=True))
                umms.append(nc.tensor.matmul(st[:, h, 1, :],
                                             k2_b[:, h, 128:256], vb[:, h, b, :],
                                             start=False, stop=False,
                                             skip_group_check=True))
                for rb in range(4):
                    umms.append(nc.tensor.matmul(st[32 * rb : 32 * rb + D1, h, 2, :],
                                                 kb[:, h, b, 0:D1], vb[:, h, b, :],
                                                 start=False, stop=False,
                                                 skip_group_check=True,
                                                 tile_position=(0, 32 * rb)))
            if prev_snap_op is not None:
                for m in umms:
                    add_dep_helper(m.ins, prev_snap_op.ins,
                                   reason="state WAR after snapshot")

            # --- Pool: snapshot state; tot copy ---
            if b < NCH - 1:
                snap = snapp.tile([128, HPC, 3, D1], bt, tag="snap")
                cp = nc.scalar.copy(snap[:], st[:])
                for m in umms:
                    add_dep_helper(cp.ins, m.ins, reason="snapshot after update")
                snap_prev = snap
                prev_snap_op = cp
            nc.scalar.copy(tot[:, :, b, :], num[:])
            if (b + 1) % (NCH // 4) == 0:
                hb = b // (NCH // 4)
                csb = slice(hb * (NCH // 4), (hb + 1) * (NCH // 4))
                rec = bulk.tile([128, HPC, NCH // 4, 1], dt, tag=f"rec{hb}")
                nc.vector.reciprocal(rec[:], tot[:, :, csb, D : D + 1])
                rec_bc = _ap(rec[:], rec[:],
                             [[NCH // 4, HPC], [1, NCH // 4], [0, D]])
                nc.vector.tensor_tensor(o_sb[:, :, csb], tot[:, :, csb, 0:D],
                                        rec_bc, op=Alu.mult)
                for h in range(HPC):
                    o_r = o_d[h].rearrange("(c p) d -> p c d", p=128)
                    nc.sync.dma_start(o_r[:, csb], o_sb[:, h, csb])

    # ---- epilogue (emitted per half from the loop): nothing left here ----


def build_program():
    nc = bacc.Bacc("TRN2", target_bir_lowering=False, debug=False)
    q_d = nc.dram_tensor("q", [HPC, S, D], dt, kind="ExternalInput")
    k_d = nc.dram_tensor("k", [HPC, S, D], dt, kind="ExternalInput")
    v_d = nc.dram_tensor("v", [HPC, S, D], dt, kind="ExternalInput")
    o_d = nc.dram_tensor("out", [HPC, S, D], dt, kind="ExternalOutput")

    with tile.TileContext(nc) as tc, ExitStack() as ctx:
        constp = ctx.enter_context(tc.tile_pool(name="const", bufs=1))
        bulk = ctx.enter_context(tc.tile_pool(name="bulk", bufs=1))
        sb = ctx.enter_context(tc.tile_pool(name="sb", bufs=3))
        snapp = ctx.enter_context(tc.tile_pool(name="snap", bufs=2))
        ps_stp = ctx.enter_context(tc.tile_pool(name="ps_stp", bufs=2, space="PSUM"))
        ps_q2t = ctx.enter_context(tc.tile_pool(name="ps_q2t", bufs=2, space="PSUM"))
        ps_num = ctx.enter_context(tc.tile_pool(name="ps_num", bufs=2, space="PSUM"))
        ps_state = ctx.enter_context(tc.tile_pool(name="ps_st", bufs=1, space="PSUM"))

        from concourse.masks import make_identity
        ident = constp.tile([128, 128], bt)
        make_identity(nc, ident)
        trih = constp.tile([128, 128], bt)
        make_upper_triangular(nc, trih, val=0.5, diag=True)
        mask = constp.tile([128, 128], bt)
        make_upper_triangular(nc, mask, val=1.0, diag=True)

        pools = ((ident, trih, mask), bulk, sb, snapp, ps_stp, ps_q2t, ps_num, ps_state)
        _build_core(nc, pools, q_d, k_d, v_d, o_d)

    nc.compile()
    return nc


_NC = None


def kernel(q: np.ndarray, k: np.ndarray, v: np.ndarray) -> np.ndarray:
    global _NC
    if _NC is None:
        _NC = build_program()
    q = np.ascontiguousarray(np.asarray(q, dtype=np.float32).reshape(H, S, D))
    k = np.ascontiguousarray(np.asarray(k, dtype=np.float32).reshape(H, S, D))
    v = np.ascontiguousarray(np.asarray(v, dtype=np.float32).reshape(H, S, D))
    in_maps = []
    for i in range(NCORES):
        sl = slice(i * HPC, (i + 1) * HPC)
        in_maps.append({
            "q": np.ascontiguousarray(q[sl]),
            "k": np.ascontiguousarray(k[sl]),
            "v": np.ascontiguousarray(v[sl]),
        })
    res = run_bass_kernel_spmd(_NC, in_maps, core_ids=list(range(NCORES)))
    outs = [res.results[i]["out"] for i in range(NCORES)]
    return np.concatenate(outs, axis=0).reshape(B, H, S, D)


# revision 52
# speedup vs baseline: 1.8817x; 1.2158x over previous
"""Trainium2 Bass kernel: Based linear attention (poly feature map, causal, normalized).

Full inputs q,k,v: [1, 16, 4096, 16] fp32. Output: [1, 16, 4096, 16] fp32.
Sharding: 16 heads over 8 cores (2 heads/core); each head is independent.

Algorithm (per head): chunked quadratic-state linear attention, C=128.
  P = 1 + s + 0.5 s^2 with s = u.k, u = q/sqrt(D).
  Intra chunk: stp[j,i] = [1|k_j].[1|u_i] = 1+s ; P = Square(stp/sqrt2) masked
  (j<=i) plus 0.5-masked (trih matmul). Cross chunk, explicit quadratic
  features: q2[i,(p,r)] = 0.5 u_p u_r (PE-transposed to [f,i]),
  k2[j,(p,r)] = k_p k_r; states M2[f,d'] = sum k2^T v', M1[e,d'] = sum
  [1|k]^T v' accumulate in PSUM; numerator = intra matmuls + q2t^T @ M2 +
  [1|u]^T-read of M1, all accumulated in one PSUM tile (v' = [v|1] carries
  the normalizer z in channel 16). Normalization batched at the end.
  k/a transposed feature tiles come from DMA xbar transposes.
"""
import numpy as np
from contextlib import ExitStack

import concourse.bass as bass
import concourse.bacc as bacc
import concourse.tile as tile
import concourse.mybir as mybir
from bass_rust import add_dep_helper
from concourse.masks import make_upper_triangular
from concourse.bass_utils import run_bass_kernel_spmd

B, H, S, D = 1, 16, 4096, 16
NCORES = 8
HPC = H // NCORES  # heads per core (2)
C = 128            # chunk (positions)
NCH = S // C       # 32 chunks
D1 = D + 1         # 17
F2 = 256           # quadratic features (p,r)
FP = 32            # padded feature width for xbar transpose (NCH*FP % 128 == 0, FP % 16 == 0)
dt = mybir.dt.float32
bt = mybir.dt.bfloat16
SCALE = 1.0 / np.sqrt(D)
RT2I = 1.0 / np.sqrt(2.0)
Alu = mybir.AluOpType
Act = mybir.ActivationFunctionType


def _ap(base_ap, offset_ap, dims):
    """AP on the same tensor as `base_ap`: partition dim kept, free dims replaced."""
    return bass.AP(tensor=base_ap.tensor, offset=offset_ap.offset,
                   ap=[base_ap.ap[0]] + dims)


def _build_core(nc, pools, q_d, k_d, v_d, o_d):
    (ident, trih, mask, wq2), bulk, sb, sbb, snapp, ps_stp, ps_q2t, ps_num, ps_state, ps_kt = pools

    # ---- raw loads (both heads): [h, S, D] -> [128, h, NCH, D] ----
    qraw = bulk.tile([128, HPC, NCH, D], dt, tag="qraw")
    kraw = bulk.tile([128, HPC, NCH, D], dt, tag="kraw")
    vraw = bulk.tile([128, HPC, NCH, D], dt, tag="vraw")
    NQ = NCH // 2

    # loads/prep/xbar are interleaved below (per half)
    # ---- feature tensors ----
    # kb = [1|k] (17 used, padded to 20 for the DMA xbar transpose), ab = [1|u]
    kb = bulk.tile([128, HPC, NCH, FP], bt, tag="kb")
    ab = bulk.tile([128, HPC, NCH, FP], bt, tag="ab")
    vb = bulk.tile([128, HPC, NCH, D1], bt, tag="vb")
    nc.vector.memset(kb[:, :, :, 0:1], 1.0)
    nc.vector.memset(ab[:, :, :, 0:1], 1.0)
    nc.vector.memset(vb[:, :, :, D : D + 1], 1.0)
    nc.vector.memset(kb[:, :, :, D1:FP], 0.0)
    nc.vector.memset(ab[:, :, :, D1:FP], 0.0)

    # real xbar layout: out[r, g, j] = in[j, 128*g + r]; with FP=32 each
    # 128-col group g packs 4 chunks at partition bases {0,32,64,96}.
    ktp = bulk.tile([128, HPC, NCH // 4, 128], bt, tag="ktp")
    atp = bulk.tile([128, HPC, NCH // 4, 128], bt, tag="atp")
    NQ4 = NCH // 2
    NG4 = NCH // 2 // 4  # xbar groups per load half

    def emit_load(qt):
        cs = slice(qt * NQ4, (qt + 1) * NQ4)
        for h in range(HPC):
            k_r = k_d[h].rearrange("(c p) d -> p c d", p=128)
            q_r = q_d[h].rearrange("(c p) d -> p c d", p=128)
            nc.sync.dma_start(kraw[:, h, cs], k_r[:, cs])
            nc.sync.dma_start(qraw[:, h, cs], q_r[:, cs])

    def emit_vload(qt):
        cs = slice(qt * NQ4, (qt + 1) * NQ4)
        for h in range(HPC):
            v_r = v_d[h].rearrange("(c p) d -> p c d", p=128)
            nc.sync.dma_start(vraw[:, h, cs], v_r[:, cs])

    def emit_prep(qt, c_lo=None, c_hi=None, g_lo=None, g_hi=None):
        c_lo = qt * NQ4 if c_lo is None else c_lo
        c_hi = (qt + 1) * NQ4 if c_hi is None else c_hi
        g_lo = c_lo // 4 if g_lo is None else g_lo
        g_hi = c_hi // 4 if g_hi is None else g_hi
        cs = slice(c_lo, c_hi)
        nc.scalar.copy(kb[:, :, cs, 1 : D + 1], kraw[:, :, cs])
        nc.scalar.mul(ab[:, :, cs, 1 : D + 1], qraw[:, :, cs], SCALE)
        if g_hi > g_lo:
            gs = slice(g_lo, g_hi)
            xcs = slice(g_lo * 4, g_hi * 4)
            for h in range(HPC):
                nc.sync.dma_start_transpose(ktp[:, h, gs], kb[:, h, xcs])
                nc.sync.dma_start_transpose(atp[:, h, gs], ab[:, h, xcs])

    def emit_pe_transp(g):
        # PE transposes for group g (4 chunks) into ktp/atp, copies on DVE
        for h in range(HPC):
            tp = ps_kt.tile([128, 2, 128], bt, tag="kt_ps")
            kb_slab = _ap(kb[:], kb[:, h, 4 * g, 0:1], [[1, 128]])
            ab_slab = _ap(ab[:], ab[:, h, 4 * g, 0:1], [[1, 128]])
            nc.tensor.matmul(tp[:, 0, :], kb_slab, ident[:], start=True,
                             stop=True, skip_group_check=True, is_transpose=True)
            nc.tensor.matmul(tp[:, 1, :], ab_slab, ident[:], start=True,
                             stop=True, skip_group_check=True, is_transpose=True)
            nc.vector.tensor_copy(ktp[:, h, g, :], tp[:, 0, :])
            nc.vector.tensor_copy(atp[:, h, g, :], tp[:, 1, :])

    def emit_vprep(qt):
        cs = slice(qt * NQ4, (qt + 1) * NQ4)
        nc.scalar.copy(vb[:, :, cs, 0:D], vraw[:, :, cs])

    # all loads issued first (independent, no SP blocking); chunks 0-7
    # prepped + PE-transposed (fast path); groups 2-3 via xbar; half-1
    # prep mid-loop.
    emit_load(0)
    emit_vload(0)
    emit_load(1)
    emit_vload(1)
    emit_prep(0, c_lo=0, c_hi=8, g_lo=0, g_hi=0)
    emit_pe_transp(0)
    emit_pe_transp(1)
    emit_vprep(0)
    emit_prep(0, c_lo=8, c_hi=16, g_lo=2, g_hi=4)

    tot = bulk.tile([128, HPC, NCH, D1], dt, tag="tot")
    o_sb = bulk.tile([128, HPC, NCH, D], dt, tag="osb")

    # persistent PSUM state: [128, h, 3, 17] = (M2a, M2b, M1)
    st = ps_state.tile([128, HPC, 3, D1], dt, tag="st")
    nc.vector.memset(st[:], 0.0)

    snap_prev = None
    prev_snap_op = None
    q2t_sb_prev = None
    pt_prev = None
    cdata = {}

    # software pipeline: at iteration c, issue front-end for chunk c
    # (builds, transposes, scores, square, mask, q2t copy) and back-end for
    # chunk c-1 (readout matmuls, state update, snapshot, tot copy).
    for c in range(NCH + 1):
        if c == 6:
            emit_prep(1)
        if c == 12:
            emit_vprep(1)
        if c < NCH:
            # --- quadratic feature builds, batched over 4 chunks (4D TT) ---
            if c % 4 == 0:
                q2b = sbb.tile([128, HPC, 4, 192], bt, tag="q2b")
                k2b = sbb.tile([128, HPC, 4, 192], bt, tag="k2b")
                for h in range(HPC):
                    a0 = _ap(ab[:], ab[:, h, c, 1:2], [[FP, 4], [0, D], [1, 8]])
                    a1 = _ap(ab[:], ab[:, h, c, 1:2], [[FP, 4], [1, D], [0, 8]])
                    nc.vector.tensor_mul(q2b[:, h, :, 0:128], a0, a1)
                    a2 = _ap(ab[:], ab[:, h, c, 9:10], [[FP, 4], [1, 8], [0, 8]])
                    a3 = _ap(ab[:], ab[:, h, c, 9:10], [[FP, 4], [0, 8], [1, 8]])
                    nc.vector.tensor_mul(q2b[:, h, :, 128:192], a2, a3)
                    kk0 = _ap(kb[:], kb[:, h, c, 1:2], [[FP, 4], [0, D], [1, 8]])
                    kk1 = _ap(kb[:], kb[:, h, c, 1:2], [[FP, 4], [1, D], [0, 8]])
                    nc.gpsimd.tensor_mul(k2b[:, h, :, 0:128], kk0, kk1)
                    kk2 = _ap(kb[:], kb[:, h, c, 9:10], [[FP, 4], [1, 8], [0, 8]])
                    kk3 = _ap(kb[:], kb[:, h, c, 9:10], [[FP, 4], [0, 8], [1, 8]])
                    nc.gpsimd.tensor_mul(k2b[:, h, :, 128:192], kk2, kk3)
                cur_q2b, cur_k2b = q2b, k2b
            q2 = cur_q2b[:, :, c % 4]
            k2 = cur_k2b[:, :, c % 4]

            # --- PE: intra scores (first: feeds Square -> mask chain) ---
            stp = ps_stp.tile([128, HPC, 128], dt, tag="stp")
            g, p0 = c // 4, 32 * (c % 4)
            for h in range(HPC):
                nc.tensor.matmul(stp[:, h, :], ktp[p0 : p0 + D1, h, g, :],
                                 atp[p0 : p0 + D1, h, g, :],
                                 start=True, stop=True, skip_group_check=True,
                                 tile_position=(p0, 0))

            # --- PE: q2 transposes -> bf16 PSUM (2-chunk shared tile) ---
            if c % 2 == 0:
                q2t_ps = ps_q2t.tile([128, 2, HPC, 2, 128], bt, tag="q2t")
                cur_q2t_ps = q2t_ps
            for h in range(HPC):
                nc.tensor.matmul(cur_q2t_ps[:, c % 2, h, 0, :],
                                 q2[:, h, 0:128], ident[:], start=True,
                                 stop=True, skip_group_check=True,
                                 is_transpose=True)
                nc.tensor.matmul(cur_q2t_ps[0:64, c % 2, h, 1, :],
                                 q2[:, h, 128:192], ident[:], start=True,
                                 stop=True, skip_group_check=True,
                                 is_transpose=True)

            # --- Act: Square; DVE: causal mask then q2t copy (x0.5) ---
            sq = sb.tile([128, HPC, 128], bt, tag="sq")
            nc.scalar.activation(sq[:], stp[:], Act.Square, scale=RT2I)
            pt = sb.tile([128, HPC, 128], bt, tag="pt")
            mask_bc = _ap(mask[:], mask[:], [[0, HPC], [1, 128]])
            nc.vector.tensor_mul(pt[:], sq[:], mask_bc)
            if c % 2 == 1:
                q2t_sb2 = sb.tile([128, 2, HPC, 2, 128], bt, tag="q2tsb")
                nc.vector.tensor_scalar_mul(q2t_sb2[:], cur_q2t_ps[:],
                                            wq2[:, 0:1])
                cdata[c - 1] = cdata[c - 1][:2] + (q2t_sb2[:, 0],) + cdata[c - 1][3:]
                q2t_sb = q2t_sb2[:, 1]
            else:
                q2t_sb = None
            cdata[c] = (q2, k2, q2t_sb, pt)

        b = c - 1
        if b >= 0:
            q2_b, k2_b, q2t_b, pt_b = cdata.pop(b)
            # --- PE: readout matmuls -> num PSUM ---
            num = ps_num.tile([128, HPC, D1], dt, tag="num")
            for h in range(HPC):
                mms = []
                mms.append(nc.tensor.matmul(num[:, h, :], pt_b[:, h, :],
                                            vb[:, h, b, :], start=True,
                                            stop=False))
                mms.append(nc.tensor.matmul(num[:, h, :], trih[:],
                                            vb[:, h, b, :], start=False,
                                            stop=(b == 0)))
                if b > 0:
                    mms.append(nc.tensor.matmul(num[:, h, :],
                                                q2t_b[:, h, 0, :],
                                                snap_prev[:, h, 0, :],
                                                start=False, stop=False))
                    mms.append(nc.tensor.matmul(num[:, h, :],
                                                q2t_b[0:64, h, 1, :],
                                                snap_prev[0:64, h, 1, :],
                                                start=False, stop=False))
                    gb, pb = b // 4, 32 * (b % 4)
                    mms.append(nc.tensor.matmul(num[:, h, :],
                                                atp[pb : pb + D1, h, gb, :],
                                                snap_prev[pb : pb + D1, h, 2, :],
                                                start=False, stop=True,
                                                tile_position=(pb, 0)))
                for m0, m1 in zip(mms, mms[1:]):
                    add_dep_helper(m1.ins, m0.ins, reason="num accum order")

            # --- PE: state update (after previous snapshot read) ---
            umms = []
            for h in range(HPC) if b < NCH - 1 else []:
                umms.append(nc.tensor.matmul(st[:, h, 0, :],
                                             k2_b[:, h, 0:128], vb[:, h, b, :],
                                             start=False, stop=False,
                                             skip_group_check=True))
                umms.append(nc.tensor.matmul(st[0:64, h, 1, :],
                                             k2_b[:, h, 128:192], vb[:, h, b, :],
                                             start=False, stop=False,
                                             skip_group_check=True))
                for rb in range(4):
                    umms.append(nc.tensor.matmul(st[32 * rb : 32 * rb + D1, h, 2, :],
                                                 kb[:, h, b, 0:D1], vb[:, h, b, :],
                                                 start=False, stop=False,
                                                 skip_group_check=True,
                                                 tile_position=(0, 32 * rb)))
            if prev_snap_op is not None:
                for m in umms:
                    add_dep_helper(m.ins, prev_snap_op.ins,
                                   reason="state WAR after snapshot")

            # --- Pool: snapshot state; tot copy ---
            if b < NCH - 1:
                snap = snapp.tile([128, HPC, 3, D1], bt, tag="snap")
                cp = nc.scalar.copy(snap[:], st[:])
                for m in umms:
                    add_dep_helper(cp.ins, m.ins, reason="snapshot after update")
                snap_prev = snap
                prev_snap_op = cp
            nc.scalar.copy(tot[:, :, b, :], num[:])
            if (b + 1) % (NCH // 4) == 0:
                hb = b // (NCH // 4)
                csb = slice(hb * (NCH // 4), (hb + 1) * (NCH // 4))
                rec = bulk.tile([128, HPC, NCH // 4, 1], dt, tag=f"rec{hb}")
                nc.vector.reciprocal(rec[:], tot[:, :, csb, D : D + 1])
                rec_bc = _ap(rec[:], rec[:],
                             [[NCH // 4, HPC], [1, NCH // 4], [0, D]])
                if hb < 3:
                    nc.gpsimd.tensor_mul(o_sb[:, :, csb], tot[:, :, csb, 0:D],
                                         rec_bc)
                else:
                    nc.vector.tensor_tensor(o_sb[:, :, csb], tot[:, :, csb, 0:D],
                                            rec_bc, op=Alu.mult)
                for h in range(HPC):
                    o_r = o_d[h].rearrange("(c p) d -> p c d", p=128)
                    nc.sync.dma_start(o_r[:, csb], o_sb[:, h, csb])

    # ---- epilogue (emitted per half from the loop): nothing left here ----


def build_program():
    nc = bacc.Bacc("TRN2", target_bir_lowering=False, debug=False)
    q_d = nc.dram_tensor("q", [HPC, S, D], dt, kind="ExternalInput")
    k_d = nc.dram_tensor("k", [HPC, S, D], dt, kind="ExternalInput")
    v_d = nc.dram_tensor("v", [HPC, S, D], dt, kind="ExternalInput")
    o_d = nc.dram_tensor("out", [HPC, S, D], dt, kind="ExternalOutput")

    with tile.TileContext(nc) as tc, ExitStack() as ctx:
        constp = ctx.enter_context(tc.tile_pool(name="const", bufs=1))
        bulk = ctx.enter_context(tc.tile_pool(name="bulk", bufs=1))
        sb = ctx.enter_context(tc.tile_pool(name="sb", bufs=12))
        sbb = ctx.enter_context(tc.tile_pool(name="sbb", bufs=3))
        snapp = ctx.enter_context(tc.tile_pool(name="snap", bufs=2))
        ps_stp = ctx.enter_context(tc.tile_pool(name="ps_stp", bufs=2, space="PSUM"))
        ps_q2t = ctx.enter_context(tc.tile_pool(name="ps_q2t", bufs=2, space="PSUM"))
        ps_num = ctx.enter_context(tc.tile_pool(name="ps_num", bufs=2, space="PSUM"))
        ps_state = ctx.enter_context(tc.tile_pool(name="ps_st", bufs=1, space="PSUM"))
        ps_kt = ctx.enter_context(tc.tile_pool(name="ps_kt", bufs=1, space="PSUM"))

        from concourse.masks import make_identity
        ident = constp.tile([128, 128], bt)
        make_identity(nc, ident)
        trih = constp.tile([128, 128], bt)
        make_upper_triangular(nc, trih, val=0.5, diag=True)
        wq2 = constp.tile([128, 1], dt)
        nc.vector.memset(wq2[0:64], 0.5)
        nc.vector.memset(wq2[64:128], 1.0)
        mask = constp.tile([128, 128], bt)
        make_upper_triangular(nc, mask, val=1.0, diag=True)

        pools = ((ident, trih, mask, wq2), bulk, sb, sbb, snapp, ps_stp, ps_q2t, ps_num, ps_state, ps_kt)
        _build_core(nc, pools, q_d, k_d, v_d, o_d)

    nc.compile()
    return nc


_NC = None


def kernel(q: np.ndarray, k: np.ndarray, v: np.ndarray) -> np.ndarray:
    global _NC
    if _NC is None:
        _NC = build_program()
    q = np.ascontiguousarray(np.asarray(q, dtype=np.float32).reshape(H, S, D))
    k = np.ascontiguousarray(np.asarray(k, dtype=np.float32).reshape(H, S, D))
    v = np.ascontiguousarray(np.asarray(v, dtype=np.float32).reshape(H, S, D))
    in_maps = []
    for i in range(NCORES):
        sl = slice(i * HPC, (i + 1) * HPC)
        in_maps.append({
            "q": np.ascontiguousarray(q[sl]),
            "k": np.ascontiguousarray(k[sl]),
            "v": np.ascontiguousarray(v[sl]),
        })
    res = run_bass_kernel_spmd(_NC, in_maps, core_ids=list(range(NCORES)))
    outs = [res.results[i]["out"] for i in range(NCORES)]
    return np.concatenate(outs, axis=0).reshape(B, H, S, D)


# revision 55
# speedup vs baseline: 1.9804x; 1.0525x over previous
"""Trainium2 Bass kernel: Based linear attention (poly feature map, causal, normalized).

Full inputs q,k,v: [1, 16, 4096, 16] fp32. Output: [1, 16, 4096, 16] fp32.
Sharding: 16 heads over 8 cores (2 heads/core); each head is independent.

Algorithm (per head): chunked quadratic-state linear attention, C=128.
  P = 1 + s + 0.5 s^2 with s = u.k, u = q/sqrt(D).
  Intra chunk: stp[j,i] = [1|k_j].[1|u_i] = 1+s ; P = Square(stp/sqrt2) masked
  (j<=i) plus 0.5-masked (trih matmul). Cross chunk, explicit quadratic
  features: q2[i,(p,r)] = 0.5 u_p u_r (PE-transposed to [f,i]),
  k2[j,(p,r)] = k_p k_r; states M2[f,d'] = sum k2^T v', M1[e,d'] = sum
  [1|k]^T v' accumulate in PSUM; numerator = intra matmuls + q2t^T @ M2 +
  [1|u]^T-read of M1, all accumulated in one PSUM tile (v' = [v|1] carries
  the normalizer z in channel 16). Normalization batched at the end.
  k/a transposed feature tiles come from DMA xbar transposes.
"""
import numpy as np
from contextlib import ExitStack

import concourse.bass as bass
import concourse.bacc as bacc
import concourse.tile as tile
import concourse.mybir as mybir
from bass_rust import add_dep_helper
from concourse.masks import make_upper_triangular
from concourse.bass_utils import run_bass_kernel_spmd

B, H, S, D = 1, 16, 4096, 16
NCORES = 8
HPC = H // NCORES  # heads per core (2)
C = 128            # chunk (positions)
NCH = S // C       # 32 chunks
D1 = D + 1         # 17
F2 = 256           # quadratic features (p,r)
FP = 32            # padded feature width for xbar transpose (NCH*FP % 128 == 0, FP % 16 == 0)
dt = mybir.dt.float32
bt = mybir.dt.bfloat16
SCALE = 1.0 / np.sqrt(D)
RT2I = 1.0 / np.sqrt(2.0)
Alu = mybir.AluOpType
Act = mybir.ActivationFunctionType


def _ap(base_ap, offset_ap, dims):
    """AP on the same tensor as `base_ap`: partition dim kept, free dims replaced."""
    return bass.AP(tensor=base_ap.tensor, offset=offset_ap.offset,
                   ap=[base_ap.ap[0]] + dims)


def _build_core(nc, pools, q_d, k_d, v_d, o_d):
    (ident, trih, mask, wq2), bulk, sb, sbb, snapp, ps_stp, ps_q2t, ps_num, ps_state, ps_kt = pools

    # ---- raw loads (both heads): [h, S, D] -> [128, h, NCH, D] ----
    qraw = bulk.tile([128, HPC, NCH, D], dt, tag="qraw")
    kraw = bulk.tile([128, HPC, NCH, D], dt, tag="kraw")
    vraw = bulk.tile([128, HPC, NCH, D], dt, tag="vraw")
    NQ = NCH // 2

    # loads/prep/xbar are interleaved below (per half)
    # ---- feature tensors ----
    # kb = [1|k] (17 used, padded to 20 for the DMA xbar transpose), ab = [1|u]
    kb = bulk.tile([128, HPC, NCH, FP], bt, tag="kb")
    ab = bulk.tile([128, HPC, NCH, FP], bt, tag="ab")
    vb = bulk.tile([128, HPC, NCH, D1], bt, tag="vb")
    warm = bulk.tile([1, 1], dt, tag="actwarm")
    nc.vector.memset(warm[:], 1.0)
    nc.scalar.activation(warm[:], warm[:], Act.Square)
    nc.vector.memset(kb[:, :, :, 0:1], 1.0)
    nc.vector.memset(ab[:, :, :, 0:1], 1.0)
    nc.vector.memset(vb[:, :, :, D : D + 1], 1.0)
    nc.vector.memset(kb[:, :, :, D1:FP], 0.0)
    nc.vector.memset(ab[:, :, :, D1:FP], 0.0)

    # real xbar layout: out[r, g, j] = in[j, 128*g + r]; with FP=32 each
    # 128-col group g packs 4 chunks at partition bases {0,32,64,96}.
    ktp = bulk.tile([128, HPC, NCH // 4, 128], bt, tag="ktp")
    atp = bulk.tile([128, HPC, NCH // 4, 128], bt, tag="atp")
    NQ4 = NCH // 2
    NG4 = NCH // 2 // 4  # xbar groups per load half

    def emit_load(qt):
        cs = slice(qt * NQ4, (qt + 1) * NQ4)
        for h in range(HPC):
            k_r = k_d[h].rearrange("(c p) d -> p c d", p=128)
            q_r = q_d[h].rearrange("(c p) d -> p c d", p=128)
            nc.sync.dma_start(kraw[:, h, cs], k_r[:, cs])
            nc.sync.dma_start(qraw[:, h, cs], q_r[:, cs])

    def emit_vload(qt):
        cs = slice(qt * NQ4, (qt + 1) * NQ4)
        for h in range(HPC):
            v_r = v_d[h].rearrange("(c p) d -> p c d", p=128)
            nc.sync.dma_start(vraw[:, h, cs], v_r[:, cs])

    def emit_prep(qt, c_lo=None, c_hi=None, g_lo=None, g_hi=None):
        c_lo = qt * NQ4 if c_lo is None else c_lo
        c_hi = (qt + 1) * NQ4 if c_hi is None else c_hi
        g_lo = c_lo // 4 if g_lo is None else g_lo
        g_hi = c_hi // 4 if g_hi is None else g_hi
        cs = slice(c_lo, c_hi)
        nc.scalar.copy(kb[:, :, cs, 1 : D + 1], kraw[:, :, cs])
        nc.scalar.mul(ab[:, :, cs, 1 : D + 1], qraw[:, :, cs], SCALE)
        if g_hi > g_lo:
            gs = slice(g_lo, g_hi)
            xcs = slice(g_lo * 4, g_hi * 4)
            for h in range(HPC):
                nc.sync.dma_start_transpose(ktp[:, h, gs], kb[:, h, xcs])
                nc.sync.dma_start_transpose(atp[:, h, gs], ab[:, h, xcs])

    def emit_pe_transp(g):
        # PE transposes for group g (4 chunks) into ktp/atp, 2 batched copies
        tp = ps_kt.tile([128, HPC, 2, 128], bt, tag="kt_ps")
        for h in range(HPC):
            kb_slab = _ap(kb[:], kb[:, h, 4 * g, 0:1], [[1, 128]])
            ab_slab = _ap(ab[:], ab[:, h, 4 * g, 0:1], [[1, 128]])
            nc.tensor.matmul(tp[:, h, 0, :], kb_slab, ident[:], start=True,
                             stop=True, skip_group_check=True, is_transpose=True)
            nc.tensor.matmul(tp[:, h, 1, :], ab_slab, ident[:], start=True,
                             stop=True, skip_group_check=True, is_transpose=True)
        gstr = (NCH // 4) * 128
        kdst = _ap(ktp[:], ktp[:, 0, g, 0:1], [[gstr, HPC], [1, 128]])
        ksrc = _ap(tp[:], tp[:], [[256, HPC], [1, 128]])
        nc.vector.tensor_copy(kdst, ksrc)
        adst = _ap(atp[:], atp[:, 0, g, 0:1], [[gstr, HPC], [1, 128]])
        asrc = _ap(tp[:], tp[:, 0, 1, 0:1], [[256, HPC], [1, 128]])
        nc.vector.tensor_copy(adst, asrc)

    def emit_vprep(qt):
        cs = slice(qt * NQ4, (qt + 1) * NQ4)
        nc.scalar.copy(vb[:, :, cs, 0:D], vraw[:, :, cs])

    # all loads issued first (independent, no SP blocking); chunks 0-7
    # prepped + PE-transposed (fast path); groups 2-3 via xbar; half-1
    # prep mid-loop.
    emit_load(0)
    emit_vload(0)
    emit_load(1)
    emit_vload(1)
    emit_prep(0, c_lo=0, c_hi=8, g_lo=0, g_hi=0)
    emit_pe_transp(0)
    emit_pe_transp(1)
    emit_vprep(0)
    emit_prep(0, c_lo=8, c_hi=16, g_lo=2, g_hi=4)

    tot = bulk.tile([128, HPC, NCH, D1], dt, tag="tot")
    o_sb = bulk.tile([128, HPC, NCH, D], dt, tag="osb")

    # persistent PSUM state: [128, h, 3, 17] = (M2a, M2b, M1)
    st = ps_state.tile([128, HPC, 3, D1], dt, tag="st")
    nc.vector.memset(st[:], 0.0)

    snap_prev = None
    prev_snap_op = None
    q2t_sb_prev = None
    pt_prev = None
    cdata = {}

    # software pipeline: at iteration c, issue front-end for chunk c
    # (builds, transposes, scores, square, mask, q2t copy) and back-end for
    # chunk c-1 (readout matmuls, state update, snapshot, tot copy).
    for c in range(NCH + 1):
        if c == 6:
            emit_prep(1)
        if c == 12:
            emit_vprep(1)
        if c < NCH:
            # --- quadratic feature builds, batched over 4 chunks (4D TT) ---
            if c % 4 == 0:
                q2b = sbb.tile([128, HPC, 4, 192], bt, tag="q2b")
                k2b = sbb.tile([128, HPC, 4, 192], bt, tag="k2b")
                for h in range(HPC):
                    a0 = _ap(ab[:], ab[:, h, c, 1:2], [[FP, 4], [0, D], [1, 8]])
                    a1 = _ap(ab[:], ab[:, h, c, 1:2], [[FP, 4], [1, D], [0, 8]])
                    nc.vector.tensor_mul(q2b[:, h, :, 0:128], a0, a1)
                    a2 = _ap(ab[:], ab[:, h, c, 9:10], [[FP, 4], [1, 8], [0, 8]])
                    a3 = _ap(ab[:], ab[:, h, c, 9:10], [[FP, 4], [0, 8], [1, 8]])
                    nc.vector.tensor_mul(q2b[:, h, :, 128:192], a2, a3)
                    kk0 = _ap(kb[:], kb[:, h, c, 1:2], [[FP, 4], [0, D], [1, 8]])
                    kk1 = _ap(kb[:], kb[:, h, c, 1:2], [[FP, 4], [1, D], [0, 8]])
                    nc.gpsimd.tensor_mul(k2b[:, h, :, 0:128], kk0, kk1)
                    kk2 = _ap(kb[:], kb[:, h, c, 9:10], [[FP, 4], [1, 8], [0, 8]])
                    kk3 = _ap(kb[:], kb[:, h, c, 9:10], [[FP, 4], [0, 8], [1, 8]])
                    nc.gpsimd.tensor_mul(k2b[:, h, :, 128:192], kk2, kk3)
                cur_q2b, cur_k2b = q2b, k2b
            q2 = cur_q2b[:, :, c % 4]
            k2 = cur_k2b[:, :, c % 4]

            # --- PE: intra scores (first: feeds Square -> mask chain) ---
            stp = ps_stp.tile([128, HPC, 128], dt, tag="stp")
            g, p0 = c // 4, 32 * (c % 4)
            for h in range(HPC):
                nc.tensor.matmul(stp[:, h, :], ktp[p0 : p0 + D1, h, g, :],
                                 atp[p0 : p0 + D1, h, g, :],
                                 start=True, stop=True, skip_group_check=True,
                                 tile_position=(p0, 0))

            # --- PE: q2 transposes -> bf16 PSUM (2-chunk shared tile) ---
            if c % 2 == 0:
                q2t_ps = ps_q2t.tile([128, 2, HPC, 2, 128], bt, tag="q2t")
                cur_q2t_ps = q2t_ps
            for h in range(HPC):
                nc.tensor.matmul(cur_q2t_ps[:, c % 2, h, 0, :],
                                 q2[:, h, 0:128], ident[:], start=True,
                                 stop=True, skip_group_check=True,
                                 is_transpose=True)
                nc.tensor.matmul(cur_q2t_ps[0:64, c % 2, h, 1, :],
                                 q2[:, h, 128:192], ident[:], start=True,
                                 stop=True, skip_group_check=True,
                                 is_transpose=True)

            # --- Act: Square; DVE: causal mask then q2t copy (x0.5) ---
            sq = sb.tile([128, HPC, 128], bt, tag="sq")
            nc.scalar.activation(sq[:], stp[:], Act.Square, scale=RT2I)
            pt = sb.tile([128, HPC, 128], bt, tag="pt")
            mask_bc = _ap(mask[:], mask[:], [[0, HPC], [1, 128]])
            nc.vector.tensor_mul(pt[:], sq[:], mask_bc)
            if c % 2 == 1:
                q2t_sb2 = sb.tile([128, 2, HPC, 2, 128], bt, tag="q2tsb")
                nc.vector.tensor_scalar_mul(q2t_sb2[:], cur_q2t_ps[:],
                                            wq2[:, 0:1])
                cdata[c - 1] = cdata[c - 1][:2] + (q2t_sb2[:, 0],) + cdata[c - 1][3:]
                q2t_sb = q2t_sb2[:, 1]
            else:
                q2t_sb = None
            cdata[c] = (q2, k2, q2t_sb, pt)

        b = c - 1
        if b >= 0:
            q2_b, k2_b, q2t_b, pt_b = cdata.pop(b)
            # --- PE: readout matmuls -> num PSUM ---
            num = ps_num.tile([128, HPC, D1], dt, tag="num")
            for h in range(HPC):
                mms = []
                mms.append(nc.tensor.matmul(num[:, h, :], pt_b[:, h, :],
                                            vb[:, h, b, :], start=True,
                                            stop=False))
                mms.append(nc.tensor.matmul(num[:, h, :], trih[:],
                                            vb[:, h, b, :], start=False,
                                            stop=(b == 0)))
                if b > 0:
                    mms.append(nc.tensor.matmul(num[:, h, :],
                                                q2t_b[:, h, 0, :],
                                                snap_prev[:, h, 0, :],
                                                start=False, stop=False))
                    mms.append(nc.tensor.matmul(num[:, h, :],
                                                q2t_b[0:64, h, 1, :],
                                                snap_prev[0:64, h, 1, :],
                                                start=False, stop=False))
                    gb, pb = b // 4, 32 * (b % 4)
                    mms.append(nc.tensor.matmul(num[:, h, :],
                                                atp[pb : pb + D1, h, gb, :],
                                                snap_prev[pb : pb + D1, h, 2, :],
                                                start=False, stop=True,
                                                tile_position=(pb, 0)))
                for m0, m1 in zip(mms, mms[1:]):
                    add_dep_helper(m1.ins, m0.ins, reason="num accum order")

            # --- PE: state update (after previous snapshot read) ---
            umms = []
            for h in range(HPC) if b < NCH - 1 else []:
                umms.append(nc.tensor.matmul(st[:, h, 0, :],
                                             k2_b[:, h, 0:128], vb[:, h, b, :],
                                             start=False, stop=False,
                                             skip_group_check=True))
                umms.append(nc.tensor.matmul(st[0:64, h, 1, :],
                                             k2_b[:, h, 128:192], vb[:, h, b, :],
                                             start=False, stop=False,
                                             skip_group_check=True))
                for rb in range(4):
                    umms.append(nc.tensor.matmul(st[32 * rb : 32 * rb + D1, h, 2, :],
                                                 kb[:, h, b, 0:D1], vb[:, h, b, :],
                                                 start=False, stop=False,
                                                 skip_group_check=True,
                                                 tile_position=(0, 32 * rb)))
            if prev_snap_op is not None:
                for m in umms:
                    add_dep_helper(m.ins, prev_snap_op.ins,
                                   reason="state WAR after snapshot")

            # --- Pool: snapshot state; tot copy ---
            if b < NCH - 1:
                snap = snapp.tile([128, HPC, 3, D1], bt, tag="snap")
                cp = nc.scalar.copy(snap[:], st[:])
                for m in umms:
                    add_dep_helper(cp.ins, m.ins, reason="snapshot after update")
                snap_prev = snap
                prev_snap_op = cp
            nc.scalar.copy(tot[:, :, b, :], num[:])
            if (b + 1) % (NCH // 4) == 0:
                hb = b // (NCH // 4)
                csb = slice(hb * (NCH // 4), (hb + 1) * (NCH // 4))
                rec = bulk.tile([128, HPC, NCH // 4, 1], dt, tag=f"rec{hb}")
                nc.vector.reciprocal(rec[:], tot[:, :, csb, D : D + 1])
                rec_bc = _ap(rec[:], rec[:],
                             [[NCH // 4, HPC], [1, NCH // 4], [0, D]])
                if hb < 3:
                    nc.gpsimd.tensor_mul(o_sb[:, :, csb], tot[:, :, csb, 0:D],
                                         rec_bc)
                else:
                    nc.vector.tensor_tensor(o_sb[:, :, csb], tot[:, :, csb, 0:D],
                                            rec_bc, op=Alu.mult)
                for h in range(HPC):
                    o_r = o_d[h].rearrange("(c p) d -> p c d", p=128)
                    nc.sync.dma_start(o_r[:, csb], o_sb[:, h, csb])

    # ---- epilogue (emitted per half from the loop): nothing left here ----


def build_program():
    nc = bacc.Bacc("TRN2", target_bir_lowering=False, debug=False)
    q_d = nc.dram_tensor("q", [HPC, S, D], dt, kind="ExternalInput")
    k_d = nc.dram_tensor("k", [HPC, S, D], dt, kind="ExternalInput")
    v_d = nc.dram_tensor("v", [HPC, S, D], dt, kind="ExternalInput")
    o_d = nc.dram_tensor("out", [HPC, S, D], dt, kind="ExternalOutput")

    with tile.TileContext(nc) as tc, ExitStack() as ctx:
        constp = ctx.enter_context(tc.tile_pool(name="const", bufs=1))
        bulk = ctx.enter_context(tc.tile_pool(name="bulk", bufs=1))
        sb = ctx.enter_context(tc.tile_pool(name="sb", bufs=12))
        sbb = ctx.enter_context(tc.tile_pool(name="sbb", bufs=3))
        snapp = ctx.enter_context(tc.tile_pool(name="snap", bufs=2))
        ps_stp = ctx.enter_context(tc.tile_pool(name="ps_stp", bufs=2, space="PSUM"))
        ps_q2t = ctx.enter_context(tc.tile_pool(name="ps_q2t", bufs=2, space="PSUM"))
        ps_num = ctx.enter_context(tc.tile_pool(name="ps_num", bufs=2, space="PSUM"))
        ps_state = ctx.enter_context(tc.tile_pool(name="ps_st", bufs=1, space="PSUM"))
        ps_kt = ctx.enter_context(tc.tile_pool(name="ps_kt", bufs=1, space="PSUM"))

        from concourse.masks import make_identity
        ident = constp.tile([128, 128], bt)
        make_identity(nc, ident)
        trih = constp.tile([128, 128], bt)
        make_upper_triangular(nc, trih, val=0.5, diag=True)
        wq2 = constp.tile([128, 1], dt)
        nc.vector.memset(wq2[0:64], 0.5)
        nc.vector.memset(wq2[64:128], 1.0)
        mask = constp.tile([128, 128], bt)
        make_upper_triangular(nc, mask, val=1.0, diag=True)

        pools = ((ident, trih, mask, wq2), bulk, sb, sbb, snapp, ps_stp, ps_q2t, ps_num, ps_state, ps_kt)
        _build_core(nc, pools, q_d, k_d, v_d, o_d)

    nc.compile()
    return nc


_NC = None


def kernel(q: np.ndarray, k: np.ndarray, v: np.ndarray) -> np.ndarray:
    global _NC
    if _NC is None:
        _NC = build_program()
    q = np.ascontiguousarray(np.asarray(q, dtype=np.float32).reshape(H, S, D))
    k = np.ascontiguousarray(np.asarray(k, dtype=np.float32).reshape(H, S, D))
    v = np.ascontiguousarray(np.asarray(v, dtype=np.float32).reshape(H, S, D))
    in_maps = []
    for i in range(NCORES):
        sl = slice(i * HPC, (i + 1) * HPC)
        in_maps.append({
            "q": np.ascontiguousarray(q[sl]),
            "k": np.ascontiguousarray(k[sl]),
            "v": np.ascontiguousarray(v[sl]),
        })
    res = run_bass_kernel_spmd(_NC, in_maps, core_ids=list(range(NCORES)))
    outs = [res.results[i]["out"] for i in range(NCORES)]
    return np.concatenate(outs, axis=0).reshape(B, H, S, D)


# revision 61
# speedup vs baseline: 2.0284x; 1.0242x over previous
"""Trainium2 Bass kernel: Based linear attention (poly feature map, causal, normalized).

Full inputs q,k,v: [1, 16, 4096, 16] fp32. Output: [1, 16, 4096, 16] fp32.
Sharding: 16 heads over 8 cores (2 heads/core); each head is independent.

Algorithm (per head): chunked quadratic-state linear attention, C=128.
  P = 1 + s + 0.5 s^2 with s = u.k, u = q/sqrt(D).
  Intra chunk: stp[j,i] = [1|k_j].[1|u_i] = 1+s ; P = Square(stp/sqrt2) masked
  (j<=i) plus 0.5-masked (trih matmul). Cross chunk, explicit quadratic
  features: q2[i,(p,r)] = 0.5 u_p u_r (PE-transposed to [f,i]),
  k2[j,(p,r)] = k_p k_r; states M2[f,d'] = sum k2^T v', M1[e,d'] = sum
  [1|k]^T v' accumulate in PSUM; numerator = intra matmuls + q2t^T @ M2 +
  [1|u]^T-read of M1, all accumulated in one PSUM tile (v' = [v|1] carries
  the normalizer z in channel 16). Normalization batched at the end.
  k/a transposed feature tiles come from DMA xbar transposes.
"""
import numpy as np
from contextlib import ExitStack

import concourse.bass as bass
import concourse.bacc as bacc
import concourse.tile as tile
import concourse.mybir as mybir
from bass_rust import add_dep_helper
from concourse.masks import make_upper_triangular
from concourse.bass_utils import run_bass_kernel_spmd

B, H, S, D = 1, 16, 4096, 16
NCORES = 8
HPC = H // NCORES  # heads per core (2)
C = 128            # chunk (positions)
NCH = S // C       # 32 chunks
D1 = D + 1         # 17
F2 = 256           # quadratic features (p,r)
FP = 32            # padded feature width for xbar transpose (NCH*FP % 128 == 0, FP % 16 == 0)
dt = mybir.dt.float32
bt = mybir.dt.bfloat16
SCALE = 1.0 / np.sqrt(D)
RT2I = 1.0 / np.sqrt(2.0)
Alu = mybir.AluOpType
Act = mybir.ActivationFunctionType


def _ap(base_ap, offset_ap, dims):
    """AP on the same tensor as `base_ap`: partition dim kept, free dims replaced."""
    return bass.AP(tensor=base_ap.tensor, offset=offset_ap.offset,
                   ap=[base_ap.ap[0]] + dims)


def _build_core(nc, pools, q_d, k_d, v_d, o_d):
    (ident, trih, mask, wq2), bulk, sb, sbb, snapp, ps_stp, ps_q2t, ps_num, ps_state, ps_kt = pools

    # ---- raw loads (both heads): [h, S, D] -> [128, h, NCH, D] ----
    qraw = bulk.tile([128, HPC, NCH, D], dt, tag="qraw")
    kraw = bulk.tile([128, HPC, NCH, D], dt, tag="kraw")
    vraw = bulk.tile([128, HPC, NCH, D], dt, tag="vraw")
    NQ = NCH // 2

    # loads/prep/xbar are interleaved below (per half)
    # ---- feature tensors ----
    # kb = [1|k] (17 used, padded to 20 for the DMA xbar transpose), ab = [1|u]
    kb = bulk.tile([128, HPC, NCH, FP], bt, tag="kb")
    ab = bulk.tile([128, HPC, NCH, FP], bt, tag="ab")
    vb = bulk.tile([128, HPC, NCH, D1], bt, tag="vb")
    warm = bulk.tile([1, 1], dt, tag="actwarm")
    nc.vector.memset(warm[:], 1.0)
    nc.scalar.activation(warm[:], warm[:], Act.Square)
    nc.vector.memset(kb[:, :, :, 0:1], 1.0)
    nc.vector.memset(ab[:, :, :, 0:1], 1.0)
    nc.vector.memset(vb[:, :, :, D : D + 1], 1.0)
    nc.vector.memset(kb[:, :, :, D1:FP], 0.0)
    nc.vector.memset(ab[:, :, :, D1:FP], 0.0)

    # real xbar layout: out[r, g, j] = in[j, 128*g + r]; with FP=32 each
    # 128-col group g packs 4 chunks at partition bases {0,32,64,96}.
    ktp = bulk.tile([128, HPC, NCH // 4, 128], bt, tag="ktp")
    atp = bulk.tile([128, HPC, NCH // 4, 128], bt, tag="atp")
    NQ4 = NCH // 2
    NG4 = NCH // 2 // 4  # xbar groups per load half

    def emit_load(qt):
        cs = slice(qt * NQ4, (qt + 1) * NQ4)
        for h in range(HPC):
            k_r = k_d[h].rearrange("(c p) d -> p c d", p=128)
            q_r = q_d[h].rearrange("(c p) d -> p c d", p=128)
            nc.sync.dma_start(kraw[:, h, cs], k_r[:, cs])
            nc.sync.dma_start(qraw[:, h, cs], q_r[:, cs])

    def emit_vload(qt):
        cs = slice(qt * NQ4, (qt + 1) * NQ4)
        for h in range(HPC):
            v_r = v_d[h].rearrange("(c p) d -> p c d", p=128)
            nc.sync.dma_start(vraw[:, h, cs], v_r[:, cs])

    def emit_prep(qt, c_lo=None, c_hi=None, g_lo=None, g_hi=None):
        c_lo = qt * NQ4 if c_lo is None else c_lo
        c_hi = (qt + 1) * NQ4 if c_hi is None else c_hi
        g_lo = c_lo // 4 if g_lo is None else g_lo
        g_hi = c_hi // 4 if g_hi is None else g_hi
        cs = slice(c_lo, c_hi)
        nc.scalar.copy(kb[:, :, cs, 1 : D + 1], kraw[:, :, cs])
        nc.scalar.mul(ab[:, :, cs, 1 : D + 1], qraw[:, :, cs], SCALE)
        if g_hi > g_lo:
            gs = slice(g_lo, g_hi)
            xcs = slice(g_lo * 4, g_hi * 4)
            for h in range(HPC):
                nc.sync.dma_start_transpose(ktp[:, h, gs], kb[:, h, xcs])
                nc.sync.dma_start_transpose(atp[:, h, gs], ab[:, h, xcs])

    def emit_pe_transp(g):
        # PE transposes for group g (4 chunks) into ktp/atp, 2 batched copies
        tp = ps_kt.tile([128, HPC, 2, 128], bt, tag="kt_ps")
        for h in range(HPC):
            kb_slab = _ap(kb[:], kb[:, h, 4 * g, 0:1], [[1, 128]])
            ab_slab = _ap(ab[:], ab[:, h, 4 * g, 0:1], [[1, 128]])
            nc.tensor.matmul(tp[:, h, 0, :], kb_slab, ident[:], start=True,
                             stop=True, skip_group_check=True, is_transpose=True)
            nc.tensor.matmul(tp[:, h, 1, :], ab_slab, ident[:], start=True,
                             stop=True, skip_group_check=True, is_transpose=True)
        gstr = (NCH // 4) * 128
        kdst = _ap(ktp[:], ktp[:, 0, g, 0:1], [[gstr, HPC], [1, 128]])
        ksrc = _ap(tp[:], tp[:], [[256, HPC], [1, 128]])
        nc.vector.tensor_copy(kdst, ksrc)
        adst = _ap(atp[:], atp[:, 0, g, 0:1], [[gstr, HPC], [1, 128]])
        asrc = _ap(tp[:], tp[:, 0, 1, 0:1], [[256, HPC], [1, 128]])
        nc.vector.tensor_copy(adst, asrc)

    def emit_vprep(qt):
        cs = slice(qt * NQ4, (qt + 1) * NQ4)
        nc.scalar.copy(vb[:, :, cs, 0:D], vraw[:, :, cs])

    # all loads issued first (independent, no SP blocking); chunks 0-7
    # prepped + PE-transposed (fast path); groups 2-3 via xbar; half-1
    # prep mid-loop.
    emit_load(0)
    emit_vload(0)
    emit_load(1)
    emit_vload(1)
    emit_prep(0, c_lo=0, c_hi=8, g_lo=0, g_hi=0)
    emit_pe_transp(0)
    emit_pe_transp(1)
    emit_vprep(0)
    emit_prep(0, c_lo=8, c_hi=16, g_lo=2, g_hi=4)

    tot = bulk.tile([128, HPC, NCH, D1], dt, tag="tot")
    o_sb = bulk.tile([128, HPC, NCH, D], dt, tag="osb")

    # persistent PSUM state: [128, h, 3, 17] = (M2a, M2b, M1)
    st = ps_state.tile([128, HPC, 3, D1], dt, tag="st")
    nc.vector.memset(st[:], 0.0)

    snap_prev = None
    prev_snap_op = None
    q2t_sb_prev = None
    pt_prev = None
    cdata = {}

    # software pipeline: at iteration c, issue front-end for chunk c
    # (builds, transposes, scores, square, mask, q2t copy) and back-end for
    # chunk c-1 (readout matmuls, state update, snapshot, tot copy).
    for c in range(NCH + 1):
        if c == 4:
            emit_prep(1)
        if c == 7:
            emit_vprep(1)
        if c < NCH:
            # --- quadratic feature builds, batched over 4 chunks (4D TT) ---
            if c % 4 == 0:
                q2b = sbb.tile([128, HPC, 4, 192], bt, tag="q2b")
                k2b = sbb.tile([128, HPC, 4, 192], bt, tag="k2b")
                for h in range(HPC):
                    a0 = _ap(ab[:], ab[:, h, c, 1:2], [[FP, 4], [0, D], [1, 8]])
                    a1 = _ap(ab[:], ab[:, h, c, 1:2], [[FP, 4], [1, D], [0, 8]])
                    nc.vector.tensor_mul(q2b[:, h, :, 0:128], a0, a1)
                    a2 = _ap(ab[:], ab[:, h, c, 9:10], [[FP, 4], [1, 8], [0, 8]])
                    a3 = _ap(ab[:], ab[:, h, c, 9:10], [[FP, 4], [0, 8], [1, 8]])
                    nc.vector.tensor_mul(q2b[:, h, :, 128:192], a2, a3)
                    kk0 = _ap(kb[:], kb[:, h, c, 1:2], [[FP, 4], [0, D], [1, 8]])
                    kk1 = _ap(kb[:], kb[:, h, c, 1:2], [[FP, 4], [1, D], [0, 8]])
                    nc.gpsimd.tensor_mul(k2b[:, h, :, 0:128], kk0, kk1)
                    kk2 = _ap(kb[:], kb[:, h, c, 9:10], [[FP, 4], [1, 8], [0, 8]])
                    kk3 = _ap(kb[:], kb[:, h, c, 9:10], [[FP, 4], [0, 8], [1, 8]])
                    nc.gpsimd.tensor_mul(k2b[:, h, :, 128:192], kk2, kk3)
                cur_q2b, cur_k2b = q2b, k2b
            q2 = cur_q2b[:, :, c % 4]
            k2 = cur_k2b[:, :, c % 4]

            # --- PE: intra scores (first: feeds Square -> mask chain) ---
            stp = ps_stp.tile([128, HPC, 128], dt, tag="stp")
            g, p0 = c // 4, 32 * (c % 4)
            for h in range(HPC):
                nc.tensor.matmul(stp[:, h, :], ktp[p0 : p0 + D1, h, g, :],
                                 atp[p0 : p0 + D1, h, g, :],
                                 start=True, stop=True, skip_group_check=True,
                                 tile_position=(p0, 0))

            # --- PE: q2 transposes -> bf16 PSUM (2-chunk shared tile) ---
            if c % 2 == 0:
                q2t_ps = ps_q2t.tile([128, 2, HPC, 2, 128], bt, tag="q2t")
                cur_q2t_ps = q2t_ps
            for h in range(HPC):
                nc.tensor.matmul(cur_q2t_ps[:, c % 2, h, 0, :],
                                 q2[:, h, 0:128], ident[:], start=True,
                                 stop=True, skip_group_check=True,
                                 is_transpose=True)
                nc.tensor.matmul(cur_q2t_ps[0:64, c % 2, h, 1, :],
                                 q2[:, h, 128:192], ident[:], start=True,
                                 stop=True, skip_group_check=True,
                                 is_transpose=True)

            # --- Act: Square; DVE: causal mask then q2t copy (x0.5) ---
            sq = sb.tile([128, HPC, 128], bt, tag="sq")
            nc.scalar.activation(sq[:], stp[:], Act.Square, scale=RT2I)
            pt = sb.tile([128, HPC, 128], bt, tag="pt")
            mask_bc = _ap(mask[:], mask[:], [[0, HPC], [1, 128]])
            nc.vector.tensor_mul(pt[:], sq[:], mask_bc)
            if c % 2 == 1:
                q2t_sb2 = sb.tile([128, 2, HPC, 2, 128], bt, tag="q2tsb")
                nc.vector.tensor_scalar_mul(q2t_sb2[:], cur_q2t_ps[:],
                                            wq2[:, 0:1])
                cdata[c - 1] = cdata[c - 1][:2] + (q2t_sb2[:, 0],) + cdata[c - 1][3:]
                q2t_sb = q2t_sb2[:, 1]
            else:
                q2t_sb = None
            cdata[c] = (q2, k2, q2t_sb, pt)

        b = c - 1
        if b >= 0:
            q2_b, k2_b, q2t_b, pt_b = cdata.pop(b)
            # --- PE: readout matmuls -> num PSUM ---
            num = ps_num.tile([128, HPC, D1], dt, tag="num")
            for h in range(HPC):
                mms = []
                mms.append(nc.tensor.matmul(num[:, h, :], pt_b[:, h, :],
                                            vb[:, h, b, :], start=True,
                                            stop=False))
                mms.append(nc.tensor.matmul(num[:, h, :], trih[:],
                                            vb[:, h, b, :], start=False,
                                            stop=(b == 0)))
                if b > 0:
                    mms.append(nc.tensor.matmul(num[:, h, :],
                                                q2t_b[:, h, 0, :],
                                                snap_prev[:, h, 0, :],
                                                start=False, stop=False))
                    mms.append(nc.tensor.matmul(num[:, h, :],
                                                q2t_b[0:64, h, 1, :],
                                                snap_prev[0:64, h, 1, :],
                                                start=False, stop=False))
                    gb, pb = b // 4, 32 * (b % 4)
                    mms.append(nc.tensor.matmul(num[:, h, :],
                                                atp[pb : pb + D1, h, gb, :],
                                                snap_prev[pb : pb + D1, h, 2, :],
                                                start=False, stop=True,
                                                tile_position=(pb, 0)))
                for m0, m1 in zip(mms, mms[1:]):
                    add_dep_helper(m1.ins, m0.ins, reason="num accum order")

            # --- PE: state update (after previous snapshot read) ---
            umms = []
            for h in range(HPC) if b < NCH - 1 else []:
                umms.append(nc.tensor.matmul(st[:, h, 0, :],
                                             k2_b[:, h, 0:128], vb[:, h, b, :],
                                             start=False, stop=False,
                                             skip_group_check=True))
                umms.append(nc.tensor.matmul(st[0:64, h, 1, :],
                                             k2_b[:, h, 128:192], vb[:, h, b, :],
                                             start=False, stop=False,
                                             skip_group_check=True))
                for rb in range(4):
                    umms.append(nc.tensor.matmul(st[32 * rb : 32 * rb + D1, h, 2, :],
                                                 kb[:, h, b, 0:D1], vb[:, h, b, :],
                                                 start=False, stop=False,
                                                 skip_group_check=True,
                                                 tile_position=(0, 32 * rb)))
            if prev_snap_op is not None:
                for m in umms:
                    add_dep_helper(m.ins, prev_snap_op.ins,
                                   reason="state WAR after snapshot")

            # --- Pool: snapshot state; tot copy ---
            if b < NCH - 1:
                snap = snapp.tile([128, HPC, 3, D1], bt, tag="snap")
                cp = nc.scalar.copy(snap[:], st[:])
                for m in umms:
                    add_dep_helper(cp.ins, m.ins, reason="snapshot after update")
                snap_prev = snap
                prev_snap_op = cp
            nc.scalar.copy(tot[:, :, b, :], num[:])
            EPI = {7: (0, 8), 15: (8, 16), 23: (16, 24), 27: (24, 28),
                   31: (28, 32)}
            if b in EPI:
                lo, hi = EPI[b]
                w_ = hi - lo
                csb = slice(lo, hi)
                rec = bulk.tile([128, HPC, w_, 1], dt, tag=f"rec{b}")
                nc.vector.reciprocal(rec[:], tot[:, :, csb, D : D + 1])
                rec_bc = _ap(rec[:], rec[:], [[w_, HPC], [1, w_], [0, D]])
                if b < 24:
                    nc.gpsimd.tensor_mul(o_sb[:, :, csb], tot[:, :, csb, 0:D],
                                         rec_bc)
                else:
                    nc.vector.tensor_tensor(o_sb[:, :, csb], tot[:, :, csb, 0:D],
                                            rec_bc, op=Alu.mult)
                for h in range(HPC):
                    o_r = o_d[h].rearrange("(c p) d -> p c d", p=128)
                    nc.sync.dma_start(o_r[:, csb], o_sb[:, h, csb])

    # ---- epilogue (emitted per half from the loop): nothing left here ----


def build_program():
    nc = bacc.Bacc("TRN2", target_bir_lowering=False, debug=False)
    q_d = nc.dram_tensor("q", [HPC, S, D], dt, kind="ExternalInput")
    k_d = nc.dram_tensor("k", [HPC, S, D], dt, kind="ExternalInput")
    v_d = nc.dram_tensor("v", [HPC, S, D], dt, kind="ExternalInput")
    o_d = nc.dram_tensor("out", [HPC, S, D], dt, kind="ExternalOutput")

    with tile.TileContext(nc) as tc, ExitStack() as ctx:
        constp = ctx.enter_context(tc.tile_pool(name="const", bufs=1))
        bulk = ctx.enter_context(tc.tile_pool(name="bulk", bufs=1))
        sb = ctx.enter_context(tc.tile_pool(name="sb", bufs=12))
        sbb = ctx.enter_context(tc.tile_pool(name="sbb", bufs=3))
        snapp = ctx.enter_context(tc.tile_pool(name="snap", bufs=3))
        ps_stp = ctx.enter_context(tc.tile_pool(name="ps_stp", bufs=2, space="PSUM"))
        ps_q2t = ctx.enter_context(tc.tile_pool(name="ps_q2t", bufs=2, space="PSUM"))
        ps_num = ctx.enter_context(tc.tile_pool(name="ps_num", bufs=2, space="PSUM"))
        ps_state = ctx.enter_context(tc.tile_pool(name="ps_st", bufs=1, space="PSUM"))
        ps_kt = ctx.enter_context(tc.tile_pool(name="ps_kt", bufs=1, space="PSUM"))

        from concourse.masks import make_identity
        ident = constp.tile([128, 128], bt)
        make_identity(nc, ident)
        trih = constp.tile([128, 128], bt)
        make_upper_triangular(nc, trih, val=0.5, diag=True)
        wq2 = constp.tile([128, 1], dt)
        nc.vector.memset(wq2[0:64], 0.5)
        nc.vector.memset(wq2[64:128], 1.0)
        mask = constp.tile([128, 128], bt)
        make_upper_triangular(nc, mask, val=1.0, diag=True)

        pools = ((ident, trih, mask, wq2), bulk, sb, sbb, snapp, ps_stp, ps_q2t, ps_num, ps_state, ps_kt)
        _build_core(nc, pools, q_d, k_d, v_d, o_d)

    nc.compile()
    return nc


_NC = None


def kernel(q: np.ndarray, k: np.ndarray, v: np.ndarray) -> np.ndarray:
    global _NC
    if _NC is None:
        _NC = build_program()
    q = np.ascontiguousarray(np.asarray(q, dtype=np.float32).reshape(H, S, D))
    k = np.ascontiguousarray(np.asarray(k, dtype=np.float32).reshape(H, S, D))
    v = np.ascontiguousarray(np.asarray(v, dtype=np.float32).reshape(H, S, D))
    in_maps = []
    for i in range(NCORES):
        sl = slice(i * HPC, (i + 1) * HPC)
        in_maps.append({
            "q": np.ascontiguousarray(q[sl]),
            "k": np.ascontiguousarray(k[sl]),
            "v": np.ascontiguousarray(v[sl]),
        })
    res = run_bass_kernel_spmd(_NC, in_maps, core_ids=list(range(NCORES)))
    outs = [res.results[i]["out"] for i in range(NCORES)]
    return np.concatenate(outs, axis=0).reshape(B, H, S, D)
